# revision 1
# baseline (speedup 1.0000x reference)
"""AttentionalPooler Trainium2 kernel: 8-core data-parallel over batch.

Math restructuring (host side, exact algebra):
  - q = LN(queries)@Wq*scale is batch-independent -> precomputed on host, shipped
    transposed as qT[d, h*N+i].
  - LN mean-subtraction folds into column-centered weight matrices:
      LN(x)@W = rstd * (x @ center(diag(g)W))   (center = subtract column means)
    Applied to Wkv (ctx LN) and W1 (post LN).
  - Softmax denominators S_h[i] come free from a ones-augmented v matmul
    (row 64 of each head's psum); reciprocal runs 8-lane on a DMA-stacked
    [8, N] tile; 1/S is row-broadcast by GPSIMD (base-0 tiles only -- base-64
    broadcasts are broken on HW) and folded into the oin in-place multiply.
  - rstd of the ctx LN is applied via the Exp activation's per-partition scale
    (for k) and folded into the v psum->sbuf evacuation (for v).

Device layouts (per core, 4 batch items):
  x natural [j-part, c-free] for LN stats (bn_stats); PE-transpose -> xT
  [c-part, j-free] feeding the kv matmul; simT [j-part, i-free] per (jc, head);
  exp on ScalarE with per-partition rstd scale; attn-out via col-tiled pairs of
  heads -> oinT [inner-part, i-free]; Wout matmul back to natural o
  [i-part, d-free] for post-LN stats; emb via rstd-as-lhsT matmul; tiny MLP.
"""

import sys

sys.path.insert(0, "/opt/trn_rl_repo")

import numpy as np

import concourse.bacc as bacc
import concourse.mybir as mybir
import concourse.tile as tile
from concourse.masks import make_identity

B, N, D = 32, 512, 1024
H, DH = 8, 64
INNER = H * DH  # 512
PROJ = 512
MID = (D + PROJ) // 2  # 768
EPS = 1e-5
NCORES = 8
BL = B // NCORES  # 4 batch items per core
FP = mybir.dt.float32
BF = mybir.dt.bfloat16
AX = mybir.AxisListType
ALU = mybir.AluOpType
ACTF = mybir.ActivationFunctionType

P = 128
GELU_KIND = "gelu"  # "identity" for CoreSim (no Gelu in interpreter)
PROBE_SKIP_NORM = False
PROBE_DUMP = ""
NJC = N // P  # 4 j-chunks
NCC = D // P  # 8 c-chunks
NIC = N // P  # 4 i-chunks
NINC = INNER // P  # 4 inner-chunks
NMC = MID // P  # 6 mid-chunks
NPC = PROJ // P  # 4 proj-chunks


def _host_prep(inputs):
    x = np.asarray(inputs["x"], np.float32)
    pos = np.asarray(inputs["pos_encoding"], np.float32)[0]  # [N, D]
    queries = np.asarray(inputs["queries"], np.float32)
    ln_q_g = np.asarray(inputs["ln_q_g"], np.float32)
    ln_ctx_g = np.asarray(inputs["ln_ctx_g"], np.float32)
    Wq = np.asarray(inputs["Wq"], np.float32)
    Wkv = np.asarray(inputs["Wkv"], np.float32)
    Wout = np.asarray(inputs["Wout"], np.float32)
    ln_post_g = np.asarray(inputs["ln_post_g"], np.float32)
    W1 = np.asarray(inputs["W1"], np.float32)
    b1 = np.asarray(inputs["b1"], np.float32)
    W2 = np.asarray(inputs["W2"], np.float32)
    b2 = np.asarray(inputs["b2"], np.float32)

    # Batch-independent query projection (fp64 for exactness of the hoist).
    qm = queries.mean(1, keepdims=True)
    qv = queries.var(1, keepdims=True)
    qn = (queries - qm) / np.sqrt(qv + EPS) * ln_q_g
    q = (qn @ Wq) * (DH ** -0.5)  # [N, INNER]
    # qT[d, h*N + i] = q[i, h*DH + d]
    qT = np.ascontiguousarray(q.reshape(N, H, DH).transpose(2, 1, 0).reshape(DH, H * N))

    Wg = ln_ctx_g[:, None] * Wkv
    wkv_c = np.ascontiguousarray(Wg - Wg.mean(0, keepdims=True))  # [D, 2*DH]

    W1g = ln_post_g[:, None] * W1
    w1_c = np.ascontiguousarray((W1g - W1g.mean(0, keepdims=True)) / N)  # [D, MID]

    b1t = np.ascontiguousarray(b1.reshape(NMC, P).T)  # [128, 6]
    b2t = np.ascontiguousarray(b2.reshape(NPC, P).T)  # [128, 4]

    common = {
        "pos": pos,
        "qT": qT,
        "wkv": wkv_c,
        "wout": np.ascontiguousarray(Wout),
        "w1": w1_c,
        "b1t": b1t,
        "w2": np.ascontiguousarray(W2),
        "b2t": b2t,
    }
    in_maps = []
    for c in range(NCORES):
        m = dict(common)
        m["x"] = np.ascontiguousarray(x[c * BL : (c + 1) * BL].reshape(BL * N, D))
        in_maps.append(m)
    return in_maps


def build_program():
    nc = bacc.Bacc("TRN2", target_bir_lowering=False, debug=False)
    x_d = nc.dram_tensor("x", [BL * N, D], FP, kind="ExternalInput")
    pos_d = nc.dram_tensor("pos", [N, D], FP, kind="ExternalInput")
    qT_d = nc.dram_tensor("qT", [DH, H * N], FP, kind="ExternalInput")
    wkv_d = nc.dram_tensor("wkv", [D, 2 * DH], FP, kind="ExternalInput")
    wout_d = nc.dram_tensor("wout", [INNER, D], FP, kind="ExternalInput")
    w1_d = nc.dram_tensor("w1", [D, MID], FP, kind="ExternalInput")
    b1t_d = nc.dram_tensor("b1t", [P, NMC], FP, kind="ExternalInput")
    w2_d = nc.dram_tensor("w2", [MID, PROJ], FP, kind="ExternalInput")
    b2t_d = nc.dram_tensor("b2t", [P, NPC], FP, kind="ExternalInput")
    out_d = nc.dram_tensor("predT", [PROJ, BL], FP, kind="ExternalOutput")
    dbg_d = (
        nc.dram_tensor("dbg", [P, 4096], FP, kind="ExternalOutput")
        if PROBE_DUMP
        else None
    )

    from contextlib import ExitStack

    with tile.TileContext(nc) as tc, ExitStack() as ctx:
        pool = lambda name, bufs, **kw: ctx.enter_context(
            tc.tile_pool(name=name, bufs=bufs, **kw)
        )
        consts = pool("consts", 1)
        w1p = pool("w1p", 3)
        xnat_p = pool("xnat", 1)
        xT_p = pool("xT", 1)
        kv_p = pool("kv", 2)
        es_p = pool("es", 5)
        oin_p = pool("oin", 1)
        onat_p = pool("onat", 1)
        small_p = pool("small", 2)
        srow_p = pool("srow", 3)
        ost_p = pool("ostp", 2)
        s8_p = pool("s8p", 2)
        sf_p = pool("sfp", 1)
        ps_sim = pool("ps_sim", 1, space="PSUM")
        ps_sh = pool("ps_sh", 1, space="PSUM")
        ps_vout = pool("ps_vout", 1, space="PSUM")
        ps_wout = pool("ps_wout", 1, space="PSUM")
        if True:
            ident = consts.tile([P, P], FP)
            make_identity(nc, ident)
            eps_sb = consts.tile([P, 1], FP)
            nc.vector.memset(eps_sb[:, :], EPS)

            pos_sb = consts.tile([P, NJC * D], FP)
            for jc in range(NJC):
                nc.sync.dma_start(
                    pos_sb[:, jc * D : (jc + 1) * D], pos_d[jc * P : (jc + 1) * P, :]
                )
            wkv_sb = consts.tile([P, NCC * 2 * DH], FP)
            for cc in range(NCC):
                nc.sync.dma_start(
                    wkv_sb[:, cc * 128 : (cc + 1) * 128],
                    wkv_d[cc * P : (cc + 1) * P, :],
                )
            wout_sb = consts.tile([P, NINC * D], FP)
            for kc in range(NINC):
                nc.sync.dma_start(
                    wout_sb[:, kc * D : (kc + 1) * D], wout_d[kc * P : (kc + 1) * P, :]
                )
            qT_sb = consts.tile([DH, H * N], FP)
            nc.sync.dma_start(qT_sb[:, :], qT_d[:, :])
            b1t_sb = consts.tile([P, NMC], FP)
            nc.sync.dma_start(b1t_sb[:, :], b1t_d[:, :])
            b2t_sb = consts.tile([P, NPC], FP)
            nc.sync.dma_start(b2t_sb[:, :], b2t_d[:, :])
            w2_sb = consts.tile([P, NMC * PROJ], FP)
            for mc in range(NMC):
                nc.sync.dma_start(
                    w2_sb[:, mc * PROJ : (mc + 1) * PROJ],
                    w2_d[mc * P : (mc + 1) * P, :],
                )
            embT4 = consts.tile([P, BL * NCC], FP)  # col = b*NCC + cc
            h1_sb = consts.tile([P, NMC * BL], FP)
            pred_sb = consts.tile([P, NPC * BL], FP)

            for b in range(BL):
                # ---- load x natural, add pos in place ----
                xn = xnat_p.tile([P, NJC * D], FP, tag="xn")
                for jc in range(NJC):
                    nc.sync.dma_start(
                        xn[:, jc * D : (jc + 1) * D],
                        x_d[b * N + jc * P : b * N + (jc + 1) * P, :],
                    )
                for jc in range(NJC):
                    s = slice(jc * D, (jc + 1) * D)
                    nc.gpsimd.tensor_add(xn[:, s], xn[:, s], pos_sb[:, s])

                if b == 0 and PROBE_DUMP == "xn":
                    nc.sync.dma_start(dbg_d[:, :], xn[:, 0:4096])
                # ---- ctx LN stats (natural layout) ----
                rstd = small_p.tile([P, NJC], FP, tag="rstd")
                for jc in range(NJC):
                    st = small_p.tile([P, 2, 6], FP, tag="bnst")
                    for g in range(2):
                        nc.vector.bn_stats(
                            st[:, g, :],
                            xn[:, jc * D + g * 512 : jc * D + (g + 1) * 512],
                        )
                    ag = small_p.tile([P, 2], FP, tag="bnag")
                    nc.vector.bn_aggr(ag[:, :], st[:, :, :])
                    sq = small_p.tile([P, 1], FP, tag="sq")
                    nc.scalar.activation(sq[:, :], ag[:, 1:2], ACTF.Sqrt, bias=eps_sb[:, :])
                    nc.vector.reciprocal(rstd[:, jc : jc + 1], sq[:, :])

                # ---- transpose x' -> xT [c-part, j-free], feed kv matmul ----
                kvps = ps_vout.tile([P, N], FP, tag="vout")
                for chalf in range(2):
                    xT = xT_p.tile([P, 4 * N], FP, tag="xT")
                    for cc4 in range(4):
                        cc = chalf * 4 + cc4
                        pt = ps_sh.tile([P, N], FP, tag="psh")
                        for jc in range(NJC):
                            nc.tensor.transpose(
                                pt[:, jc * P : (jc + 1) * P],
                                xn[:, jc * D + cc * P : jc * D + (cc + 1) * P],
                                ident[:, :],
                            )
                        nc.scalar.copy(xT[:, cc4 * N : (cc4 + 1) * N], pt[:, :])
                    for cc4 in range(4):
                        cc = chalf * 4 + cc4
                        nc.tensor.matmul(
                            kvps[:, :],
                            wkv_sb[:, cc * 128 : (cc + 1) * 128],
                            xT[:, cc4 * N : (cc4 + 1) * N],
                            start=(cc == 0),
                            stop=(cc == NCC - 1),
                        )
                kvT = kv_p.tile([P, N], FP, tag="kvT")
                nc.vector.tensor_copy(kvT[:, :], kvps[:, :])
                if b == 0 and PROBE_DUMP == "kvT":
                    nc.sync.dma_start(dbg_d[:, 0:N], kvT[:, :])

                # ---- v natural [j-part, d-free], scaled by rstd[j] ----
                v_nat = kv_p.tile([P, NJC * (DH + 1)], BF, tag="vnat")
                for jc in range(NJC):
                    vt = ps_sh.tile([P, DH], FP, tag="psh")
                    nc.tensor.transpose(
                        vt[:, :],
                        kvT[DH:, jc * P : (jc + 1) * P],
                        ident[DH:P, DH:P],
                    )
                    nc.vector.tensor_scalar_mul(
                        v_nat[:, jc * 65 : jc * 65 + DH],
                        vt[:, :],
                        rstd[:, jc : jc + 1],
                    )
                    nc.vector.memset(v_nat[:, jc * 65 + DH : (jc + 1) * 65], 1.0)

                # ---- sim + exp per (jc, head-group of 4) ----
                es_tiles = []
                for jc in range(NJC):
                    es_t = es_p.tile([P, H * N], BF, tag="es")
                    es_tiles.append(es_t)
                    for hg in range(2):
                        sm = ps_sim.tile([P, 4 * N], FP, tag="sim")
                        for hh in range(4):
                            h = hg * 4 + hh
                            nc.tensor.matmul(
                                sm[:, hh * N : (hh + 1) * N],
                                kvT[0:DH, jc * P : (jc + 1) * P],
                                qT_sb[:, h * N : (h + 1) * N],
                                start=True,
                                stop=True,
                            )
                        nc.scalar.activation(
                            es_t[:, hg * 4 * N : (hg + 1) * 4 * N],
                            sm[:, :],
                            ACTF.Exp,
                            scale=rstd[:, jc : jc + 1],
                        )

                # ---- attn @ v with ones-aug (row 64 = softmax denom) ----
                oin = oin_p.tile([P, NINC * N], FP, tag="oin")
                s8 = s8_p.tile([8, N], FP, tag="s8")
                s8r = s8_p.tile([8, N], FP, tag="s8r")
                for h in range(H):
                    vop = ps_vout if h % 2 == 0 else ps_sh
                    vo = vop.tile([DH + 1, N], FP, tag="vout" if h % 2 == 0 else "psh")
                    for jc in range(NJC):
                        nc.tensor.matmul(
                            vo[:, :],
                            v_nat[:, jc * 65 : (jc + 1) * 65],
                            es_tiles[jc][:, h * N : (h + 1) * N],
                            start=(jc == 0),
                            stop=(jc == NJC - 1),
                        )
                    srow = srow_p.tile([P, N], FP, tag="srow")
                    nc.vector.tensor_copy(srow[DH : DH + 1, :], vo[DH : DH + 1, :])
                    nc.gpsimd.dma_start(s8[h : h + 1, :], srow[DH : DH + 1, :])
                    kc = h // 2
                    if h % 2 == 0:
                        nc.scalar.copy(
                            oin[0:DH, kc * N : (kc + 1) * N], vo[0:DH, :]
                        )
                    else:
                        ost = ost_p.tile([DH, N], FP, tag="ost")
                        nc.vector.tensor_copy(ost[:, :], vo[0:DH, :])
                        nc.gpsimd.dma_start(oin[DH:P, kc * N : (kc + 1) * N], ost[:, :])
                nc.vector.reciprocal(s8r[:, :], s8[:, :])
                sflat = sf_p.tile([1, H * N], FP, tag="sflat")
                nc.gpsimd.dma_start(
                    sflat[0:1, :].rearrange("q (h n) -> q h n", n=N), s8r[:, :]
                )
                if b == 0 and PROBE_DUMP == "s8":
                    nc.sync.dma_start(dbg_d[0:8, 0:N], s8[:, :])
                    nc.sync.dma_start(dbg_d[16:17, 0 : H * N], sflat[0:1, :])
                for kc in range(NINC):
                    rb0 = kv_p.tile([P, N], FP, tag="rb0")
                    rb1 = kv_p.tile([P, N], FP, tag="rb1")
                    nc.gpsimd.partition_broadcast(
                        rb0[:, :], sflat[0:1, 2 * kc * N : (2 * kc + 1) * N], channels=P
                    )
                    nc.gpsimd.partition_broadcast(
                        rb1[:, :], sflat[0:1, (2 * kc + 1) * N : (2 * kc + 2) * N], channels=P
                    )
                    nc.vector.tensor_mul(
                        oin[0:DH, kc * N : (kc + 1) * N],
                        oin[0:DH, kc * N : (kc + 1) * N],
                        rb0[0:DH, :],
                    )
                    nc.vector.tensor_mul(
                        oin[DH:P, kc * N : (kc + 1) * N],
                        oin[DH:P, kc * N : (kc + 1) * N],
                        rb1[DH:P, :],
                    )

                if b == 0 and PROBE_DUMP == "oin":
                    nc.sync.dma_start(dbg_d[:, 0 : NINC * N], oin[:, :])
                # ---- Wout matmul -> o natural; post-LN stats; evac ----
                onat = onat_p.tile([P, NIC * D], FP, tag="onat")
                rstdo = small_p.tile([P, NIC], FP, tag="rstdo")
                for ic in range(NIC):
                    wp = ps_wout.tile([P, D], FP, tag="wout")
                    for half in range(2):
                        for kc in range(NINC):
                            nc.tensor.matmul(
                                wp[:, half * 512 : (half + 1) * 512],
                                oin[:, kc * N + ic * P : kc * N + (ic + 1) * P],
                                wout_sb[:, kc * D + half * 512 : kc * D + half * 512 + 512],
                                start=(kc == 0),
                                stop=(kc == NINC - 1),
                            )
                    st2 = small_p.tile([P, 2, 6], FP, tag="bnst2")
                    for g in range(2):
                        nc.vector.bn_stats(
                            st2[:, g, :], wp[:, g * 512 : (g + 1) * 512]
                        )
                    ag2 = small_p.tile([P, 2], FP, tag="bnag2")
                    nc.vector.bn_aggr(ag2[:, :], st2[:, :, :])
                    sq2 = small_p.tile([P, 1], FP, tag="sq2")
                    nc.scalar.activation(sq2[:, :], ag2[:, 1:2], ACTF.Sqrt, bias=eps_sb[:, :])
                    nc.vector.reciprocal(rstdo[:, ic : ic + 1], sq2[:, :])
                    nc.vector.tensor_copy(onat[:, ic * D : (ic + 1) * D], wp[:, :])

                # ---- emb_raw = sum_i rstd_o[i] * o[i, :] (folded 1/N in w1) ----
                if b == 0 and PROBE_DUMP == "onat":
                    nc.sync.dma_start(dbg_d[:, :], onat[:, :])
                embps = ps_sh.tile([P, NCC], FP, tag="psh")
                for cc in range(NCC):
                    for ic in range(NIC):
                        nc.tensor.matmul(
                            embps[:, cc : cc + 1],
                            onat[:, ic * D + cc * P : ic * D + (cc + 1) * P],
                            rstdo[:, ic : ic + 1],
                            start=(ic == 0),
                            stop=(ic == NIC - 1),
                        )
                nc.vector.tensor_copy(
                    embT4[:, b * NCC : (b + 1) * NCC], embps[:, :]
                )

            if PROBE_DUMP == "embT4":
                nc.sync.dma_start(dbg_d[:, 0 : BL * NCC], embT4[:, :])
            # ---- MLP ----
            embT4_r = embT4.rearrange("p (b c) -> p c b", c=NCC)
            for mc in range(NMC):
                hp = ps_sh.tile([P, BL], FP, tag="psh")
                for cc in range(NCC):
                    w1t = w1p.tile([P, P], FP, tag="w1t")
                    nc.sync.dma_start(
                        w1t[:, :],
                        w1_d[cc * P : (cc + 1) * P, mc * P : (mc + 1) * P],
                    )
                    nc.tensor.matmul(
                        hp[:, :],
                        w1t[:, :],
                        embT4_r[:, cc, :],
                        start=(cc == 0),
                        stop=(cc == NCC - 1),
                    )
                gf = ACTF.Gelu if GELU_KIND == "gelu" else ACTF.Identity
                nc.scalar.activation(
                    h1_sb[:, mc * BL : (mc + 1) * BL],
                    hp[:, :],
                    gf,
                    bias=b1t_sb[:, mc : mc + 1],
                )
            for pc in range(NPC):
                pp = ps_sh.tile([P, BL], FP, tag="psh")
                for mc in range(NMC):
                    nc.tensor.matmul(
                        pp[:, :],
                        w2_sb[:, mc * PROJ + pc * P : mc * PROJ + (pc + 1) * P],
                        h1_sb[:, mc * BL : (mc + 1) * BL],
                        start=(mc == 0),
                        stop=(mc == NMC - 1),
                    )
                nc.vector.tensor_scalar_add(
                    pred_sb[:, pc * BL : (pc + 1) * BL], pp[:, :], b2t_sb[:, pc : pc + 1]
                )
                nc.sync.dma_start(
                    out_d[pc * P : (pc + 1) * P, :], pred_sb[:, pc * BL : (pc + 1) * BL]
                )

    nc.compile()
    return nc


_NC_CACHE = None


def kernel(**inputs) -> np.ndarray:
    global _NC_CACHE
    from concourse.bass_utils import run_bass_kernel_spmd

    in_maps = _host_prep(inputs)
    if _NC_CACHE is None:
        _NC_CACHE = build_program()
    nc = _NC_CACHE
    res = run_bass_kernel_spmd(nc, in_maps, core_ids=list(range(NCORES)))
    out = np.empty((B, PROJ), np.float32)
    for c in range(NCORES):
        out[c * BL : (c + 1) * BL] = res.results[c]["predT"].T
    return out



# revision 4
# speedup vs baseline: 1.7768x; 1.7768x over previous
"""AttentionalPooler Trainium2 kernel: 8-core data-parallel over batch.

Math restructuring (host side, exact algebra):
  - q = LN(queries)@Wq*scale is batch-independent -> precomputed on host, shipped
    transposed as qT[d, h*N+i] in bf16.
  - LN mean-subtraction folds into column-centered weight matrices:
      LN(x)@W = rstd * (x @ center(diag(g)W))   (center = subtract column means)
    Applied to Wkv (ctx LN) and W1 (post LN).
  - Softmax denominators S_h[i] come free from a ones-augmented v matmul
    (row 64 of each head's psum); reciprocal runs 8-lane on a DMA-stacked
    [8, N] tile; 1/S rows are expanded to [128, N] head-pair tiles by a
    K=8 selector matmul on the PE (e8 one-hot lhsT), then folded into the
    oin in-place multiply.
  - rstd of the ctx LN is applied via the Exp activation's per-partition scale
    (for k) and folded into the v psum->sbuf evacuation (for v).

v2 (speed): the whole matmul path runs bf16 (x+pos add casts to bf16; all
psum evacuations cast; weights shipped bf16) -- fp32 matmuls run at half PE
rate and drew enough power to DVFS-throttle the core 66% of the run.  MLP
weights preload to SBUF at kernel start (no serial DMA tail), x is
double-buffered across batch items, and the gpsimd partition_broadcast of
1/S (40us) is replaced by the selector matmul (~3us of PE).
"""

import sys

sys.path.insert(0, "/opt/trn_rl_repo")

import numpy as np
import ml_dtypes

import concourse.bacc as bacc
import concourse.mybir as mybir
import concourse.tile as tile
from concourse.masks import make_identity

B, N, D = 32, 512, 1024
H, DH = 8, 64
INNER = H * DH  # 512
PROJ = 512
MID = (D + PROJ) // 2  # 768
EPS = 1e-5
NCORES = 8
BL = B // NCORES  # 4 batch items per core
FP = mybir.dt.float32
BF = mybir.dt.bfloat16
AX = mybir.AxisListType
ALU = mybir.AluOpType
ACTF = mybir.ActivationFunctionType

P = 128
GELU_KIND = "gelu"  # "identity" for CoreSim (no Gelu in interpreter)
NJC = N // P  # 4 j-chunks
NCC = D // P  # 8 c-chunks
NIC = N // P  # 4 i-chunks
NINC = INNER // P  # 4 inner-chunks
NMC = MID // P  # 6 mid-chunks
NPC = PROJ // P  # 4 proj-chunks

BF_NP = ml_dtypes.bfloat16


def _host_prep(inputs):
    x = np.asarray(inputs["x"], np.float32)
    pos = np.asarray(inputs["pos_encoding"], np.float32)[0]  # [N, D]
    queries = np.asarray(inputs["queries"], np.float32)
    ln_q_g = np.asarray(inputs["ln_q_g"], np.float32)
    ln_ctx_g = np.asarray(inputs["ln_ctx_g"], np.float32)
    Wq = np.asarray(inputs["Wq"], np.float32)
    Wkv = np.asarray(inputs["Wkv"], np.float32)
    Wout = np.asarray(inputs["Wout"], np.float32)
    ln_post_g = np.asarray(inputs["ln_post_g"], np.float32)
    W1 = np.asarray(inputs["W1"], np.float32)
    b1 = np.asarray(inputs["b1"], np.float32)
    W2 = np.asarray(inputs["W2"], np.float32)
    b2 = np.asarray(inputs["b2"], np.float32)

    # Batch-independent query projection.
    qm = queries.mean(1, keepdims=True)
    qv = queries.var(1, keepdims=True)
    qn = (queries - qm) / np.sqrt(qv + EPS) * ln_q_g
    q = (qn @ Wq) * (DH ** -0.5)  # [N, INNER]
    # qT[d, h*N + i] = q[i, h*DH + d]
    qT = np.ascontiguousarray(
        q.reshape(N, H, DH).transpose(2, 1, 0).reshape(DH, H * N).astype(BF_NP)
    )

    Wg = ln_ctx_g[:, None] * Wkv
    wkv_c = np.ascontiguousarray((Wg - Wg.mean(0, keepdims=True)).astype(BF_NP))

    W1g = ln_post_g[:, None] * W1
    w1_c = np.ascontiguousarray((W1g - W1g.mean(0, keepdims=True)) / N)  # [D, MID]

    b1t = np.ascontiguousarray(b1.reshape(NMC, P).T)  # [128, 6]
    b2t = np.ascontiguousarray(b2.reshape(NPC, P).T)  # [128, 4]

    # e8[k, kc*128 + m] = 1 iff k == 2*kc + m//64 : selects 1/S rows for the
    # head-pair kc, upper/lower 64 partitions.
    e8 = np.zeros((8, NINC * P), np.float32)
    for kc in range(NINC):
        e8[2 * kc, kc * P : kc * P + DH] = 1.0
        e8[2 * kc + 1, kc * P + DH : (kc + 1) * P] = 1.0

    common = {
        "pos": pos,
        "qT": qT,
        "wkv": wkv_c,
        "wout": np.ascontiguousarray(Wout.astype(BF_NP)),
        "w1": np.ascontiguousarray(w1_c),
        "b1t": b1t,
        "w2": np.ascontiguousarray(W2),
        "b2t": b2t,
        "e8": e8,
    }
    in_maps = []
    for c in range(NCORES):
        m = dict(common)
        m["x"] = np.ascontiguousarray(x[c * BL : (c + 1) * BL].reshape(BL * N, D))
        in_maps.append(m)
    return in_maps


def build_program():
    nc = bacc.Bacc("TRN2", target_bir_lowering=False, debug=False)
    x_d = nc.dram_tensor("x", [BL * N, D], FP, kind="ExternalInput")
    pos_d = nc.dram_tensor("pos", [N, D], FP, kind="ExternalInput")
    qT_d = nc.dram_tensor("qT", [DH, H * N], BF, kind="ExternalInput")
    wkv_d = nc.dram_tensor("wkv", [D, 2 * DH], BF, kind="ExternalInput")
    wout_d = nc.dram_tensor("wout", [INNER, D], BF, kind="ExternalInput")
    w1_d = nc.dram_tensor("w1", [D, MID], FP, kind="ExternalInput")
    b1t_d = nc.dram_tensor("b1t", [P, NMC], FP, kind="ExternalInput")
    w2_d = nc.dram_tensor("w2", [MID, PROJ], FP, kind="ExternalInput")
    b2t_d = nc.dram_tensor("b2t", [P, NPC], FP, kind="ExternalInput")
    e8_d = nc.dram_tensor("e8", [8, NINC * P], FP, kind="ExternalInput")
    out_d = nc.dram_tensor("predT", [PROJ, BL], FP, kind="ExternalOutput")

    from contextlib import ExitStack

    with tile.TileContext(nc) as tc, ExitStack() as ctx:
        pool = lambda name, bufs, **kw: ctx.enter_context(
            tc.tile_pool(name=name, bufs=bufs, **kw)
        )
        consts = pool("consts", 1)
        xraw_p = pool("xraw", 2)
        xnb_p = pool("xnb", 2)
        xT_p = pool("xT", 2)
        kv_p = pool("kv", 2)
        es_p = pool("es", 4)
        oin_p = pool("oin", 2)
        ost_p = pool("ost", 2)
        srow_p = pool("srow", 2)
        s8_p = pool("s8p", 2)
        onat_p = pool("onat", 1)
        small_p = pool("small", 2)
        ps_sim = pool("ps_sim", 1, space="PSUM")
        ps_tr = pool("ps_tr", 1, space="PSUM")
        ps_mix = pool("ps_mix", 1, space="PSUM")
        ps_vo = pool("ps_vo", 2, space="PSUM")
        ps_wout = pool("ps_wout", 1, space="PSUM")

        identb = consts.tile([P, P], BF)
        make_identity(nc, identb)
        eps_sb = consts.tile([P, 1], FP)
        nc.vector.memset(eps_sb[:, :], EPS)

        pos_sb = consts.tile([P, NJC * D], FP)
        for jc in range(NJC):
            nc.sync.dma_start(
                pos_sb[:, jc * D : (jc + 1) * D], pos_d[jc * P : (jc + 1) * P, :]
            )
        wkv_sb = consts.tile([P, NCC * 2 * DH], BF)
        for cc in range(NCC):
            nc.sync.dma_start(
                wkv_sb[:, cc * 128 : (cc + 1) * 128],
                wkv_d[cc * P : (cc + 1) * P, :],
            )
        wout_sb = consts.tile([P, NINC * D], BF)
        for kc in range(NINC):
            nc.sync.dma_start(
                wout_sb[:, kc * D : (kc + 1) * D], wout_d[kc * P : (kc + 1) * P, :]
            )
        qT_sb = consts.tile([DH, H * N], BF)
        nc.sync.dma_start(qT_sb[:, :], qT_d[:, :])
        e8_sb = consts.tile([8, NINC * P], FP)
        nc.sync.dma_start(e8_sb[:, :], e8_d[:, :])
        b1t_sb = consts.tile([P, NMC], FP)
        nc.sync.dma_start(b1t_sb[:, :], b1t_d[:, :])
        b2t_sb = consts.tile([P, NPC], FP)
        nc.sync.dma_start(b2t_sb[:, :], b2t_d[:, :])
        w1_sb = consts.tile([P, NCC * MID], FP)
        for cc in range(NCC):
            nc.sync.dma_start(
                w1_sb[:, cc * MID : (cc + 1) * MID], w1_d[cc * P : (cc + 1) * P, :]
            )
        w2_sb = consts.tile([P, NMC * PROJ], FP)
        for mc in range(NMC):
            nc.sync.dma_start(
                w2_sb[:, mc * PROJ : (mc + 1) * PROJ],
                w2_d[mc * P : (mc + 1) * P, :],
            )
        embT4 = consts.tile([P, BL * NCC], FP)  # col = b*NCC + cc
        h1_sb = consts.tile([P, NMC * BL], FP)
        pred_sb = consts.tile([P, NPC * BL], FP)

        # Deferred per-b state for the emb matmul interleave.
        for b in range(BL):
            # ---- load x, add pos -> bf16 ----
            xr = xraw_p.tile([P, NJC * D], FP, tag="xr")
            for jc in range(NJC):
                nc.sync.dma_start(
                    xr[:, jc * D : (jc + 1) * D],
                    x_d[b * N + jc * P : b * N + (jc + 1) * P, :],
                )
            xnb = xnb_p.tile([P, NJC * D], BF, tag="xnb")
            for jc in range(NJC):
                s = slice(jc * D, (jc + 1) * D)
                nc.vector.tensor_add(xnb[:, s], xr[:, s], pos_sb[:, s])

            # ---- ctx LN rstd (stats on bf16; var error averages out) ----
            rstd = small_p.tile([P, NJC], FP, tag="rstd")
            for jc in range(NJC):
                st = small_p.tile([P, 2, 6], FP, tag="bnst")
                for g in range(2):
                    nc.vector.bn_stats(
                        st[:, g, :],
                        xnb[:, jc * D + g * 512 : jc * D + (g + 1) * 512],
                    )
                ag = small_p.tile([P, 2], FP, tag="bnag")
                nc.vector.bn_aggr(ag[:, :], st[:, :, :])
                sq = small_p.tile([P, 1], FP, tag="sq")
                nc.scalar.activation(sq[:, :], ag[:, 1:2], ACTF.Sqrt, bias=eps_sb[:, :])
                nc.vector.reciprocal(rstd[:, jc : jc + 1], sq[:, :])

            # ---- transpose xnb -> xT [c-part, j-free] bf16; kv matmul ----
            kvps = ps_mix.tile([P, N], FP, tag="mix")
            for chalf in range(2):
                xT = xT_p.tile([P, 4 * N], BF, tag="xT")
                for ccp in range(2):  # pairs of c-chunks share one psum bank
                    pt = ps_tr.tile([P, 2 * N], BF, tag="pt")
                    for cci in range(2):
                        cc = chalf * 4 + ccp * 2 + cci
                        for jc in range(NJC):
                            nc.tensor.transpose(
                                pt[:, cci * N + jc * P : cci * N + (jc + 1) * P],
                                xnb[:, jc * D + cc * P : jc * D + (cc + 1) * P],
                                identb[:, :],
                            )
                    dsts = xT[:, ccp * 2 * N : (ccp + 1) * 2 * N]
                    if ccp == 0:
                        nc.scalar.copy(dsts, pt[:, :])
                    else:
                        nc.vector.tensor_copy(dsts, pt[:, :])
                for cc4 in range(4):
                    cc = chalf * 4 + cc4
                    nc.tensor.matmul(
                        kvps[:, :],
                        wkv_sb[:, cc * 128 : (cc + 1) * 128],
                        xT[:, cc4 * N : (cc4 + 1) * N],
                        start=(cc == 0),
                        stop=(cc == NCC - 1),
                    )
            kvT = kv_p.tile([P, N], BF, tag="kvT")
            nc.vector.tensor_copy(kvT[:, :], kvps[:, :])

            # ---- v natural [j-part, d-free] bf16, scaled by rstd[j] ----
            v_nat = kv_p.tile([P, NJC * (DH + 1)], BF, tag="vnat")
            for jc in range(NJC):
                vt = ps_tr.tile([P, DH], BF, tag="pt")
                nc.tensor.transpose(
                    vt[:, :],
                    kvT[DH:, jc * P : (jc + 1) * P],
                    identb[DH:P, DH:P],
                )
                nc.vector.tensor_scalar_mul(
                    v_nat[:, jc * 65 : jc * 65 + DH],
                    vt[:, :],
                    rstd[:, jc : jc + 1],
                )
                nc.vector.memset(v_nat[:, jc * 65 + DH : (jc + 1) * 65], 1.0)

            # ---- sim + exp per (jc, head-pair) ----
            es_tiles = []
            for jc in range(NJC):
                es_t = es_p.tile([P, H * N], BF, tag="es")
                es_tiles.append(es_t)
                for hq in range(4):
                    sm = ps_sim.tile([P, 2 * N], FP, tag="sim")
                    for hh in range(2):
                        h = hq * 2 + hh
                        nc.tensor.matmul(
                            sm[:, hh * N : (hh + 1) * N],
                            kvT[0:DH, jc * P : (jc + 1) * P],
                            qT_sb[:, h * N : (h + 1) * N],
                            start=True,
                            stop=True,
                        )
                    nc.scalar.activation(
                        es_t[:, hq * 2 * N : (hq + 1) * 2 * N],
                        sm[:, :],
                        ACTF.Exp,
                        scale=rstd[:, jc : jc + 1],
                    )

            # ---- attn @ v with ones-aug (row 64 = softmax denom) ----
            oin = oin_p.tile([P, NINC * N], BF, tag="oin")
            s8 = s8_p.tile([8, N], FP, tag="s8")
            for h in range(H):
                vo = ps_vo.tile([DH + 1, N], FP, tag="vo")
                for jc in range(NJC):
                    nc.tensor.matmul(
                        vo[:, :],
                        v_nat[:, jc * 65 : (jc + 1) * 65],
                        es_tiles[jc][:, h * N : (h + 1) * N],
                        start=(jc == 0),
                        stop=(jc == NJC - 1),
                    )
                srow = srow_p.tile([P, N], FP, tag="srow")
                nc.vector.tensor_copy(srow[DH : DH + 1, :], vo[DH : DH + 1, :])
                nc.gpsimd.dma_start(s8[h : h + 1, :], srow[DH : DH + 1, :])
                kc = h // 2
                if h % 2 == 0:
                    nc.vector.tensor_copy(
                        oin[0:DH, kc * N : (kc + 1) * N], vo[0:DH, :]
                    )
                else:
                    ost = ost_p.tile([DH, N], BF, tag="ost")
                    nc.vector.tensor_copy(ost[:, :], vo[0:DH, :])
                    nc.gpsimd.dma_start(oin[DH:P, kc * N : (kc + 1) * N], ost[:, :])
            s8r = s8_p.tile([8, N], FP, tag="s8r")
            nc.vector.reciprocal(s8r[:, :], s8[:, :])
            s8b = s8_p.tile([8, N], BF, tag="s8b")
            nc.scalar.copy(s8b[:, :], s8r[:, :])
            for kc in range(NINC):
                rb = ps_vo.tile([P, N], FP, tag="vo")
                nc.tensor.matmul(
                    rb[:, :],
                    e8_sb[:, kc * P : (kc + 1) * P],
                    s8r[:, :],
                    start=True,
                    stop=True,
                )
                nc.vector.tensor_mul(
                    oin[:, kc * N : (kc + 1) * N],
                    oin[:, kc * N : (kc + 1) * N],
                    rb[:, :],
                )

            # ---- Wout matmul -> o natural; post-LN stats; scaled evac; emb ----
            onat = onat_p.tile([P, NIC * D], BF, tag="onat")
            rstdo = small_p.tile([P, NIC], FP, tag="rstdo")
            rstdob = small_p.tile([P, NIC], BF, tag="rstdob")
            embps = ps_mix.tile([P, NIC, NCC], FP, tag="mix")
            wps = []
            for ic in range(NIC):
                wp = ps_wout.tile([P, D], FP, tag="wout")
                wps.append(wp)
                for kc in range(NINC):
                    for half in range(2):
                        nc.tensor.matmul(
                            wp[:, half * 512 : (half + 1) * 512],
                            oin[:, kc * N + ic * P : kc * N + (ic + 1) * P],
                            wout_sb[:, kc * D + half * 512 : kc * D + half * 512 + 512],
                            start=(kc == 0),
                            stop=(kc == NINC - 1),
                        )
                st2 = small_p.tile([P, 2, 6], FP, tag="bnst2")
                for g in range(2):
                    nc.vector.bn_stats(st2[:, g, :], wp[:, g * 512 : (g + 1) * 512])
                ag2 = small_p.tile([P, 2], FP, tag="bnag2")
                nc.vector.bn_aggr(ag2[:, :], st2[:, :, :])
                sq2 = small_p.tile([P, 1], FP, tag="sq2")
                nc.scalar.activation(sq2[:, :], ag2[:, 1:2], ACTF.Sqrt, bias=eps_sb[:, :])
                nc.vector.reciprocal(rstdo[:, ic : ic + 1], sq2[:, :])
                nc.scalar.copy(rstdob[:, ic : ic + 1], rstdo[:, ic : ic + 1])
                nc.vector.tensor_copy(onat[:, ic * D : (ic + 1) * D], wp[:, :])
                # emb matmuls for the PREVIOUS ic (hides the rstdo latency
                # behind this ic's wout matmuls on the PE queue).
                if ic > 0:
                    icp = ic - 1
                    for cc in range(NCC):
                        nc.tensor.matmul(
                            embps[:, icp, cc : cc + 1],
                            onat[:, icp * D + cc * P : icp * D + (cc + 1) * P],
                            rstdob[:, icp : icp + 1],
                            start=True,
                            stop=True,
                        )
            for cc in range(NCC):
                nc.tensor.matmul(
                    embps[:, 3, cc : cc + 1],
                    onat[:, 3 * D + cc * P : 3 * D + (cc + 1) * P],
                    rstdob[:, 3:4],
                    start=True,
                    stop=True,
                )
            ebt = embT4[:, b * NCC : (b + 1) * NCC]
            nc.vector.tensor_copy(ebt, embps[:, 0, :])
            for icp in range(1, NIC):
                nc.vector.tensor_add(ebt, ebt, embps[:, icp, :])

        # ---- MLP (weights preloaded in SBUF) ----
        embT4_r = embT4.rearrange("p (b c) -> p c b", c=NCC)
        for mc in range(NMC):
            hp = ps_mix.tile([P, BL], FP, tag="mix")
            for cc in range(NCC):
                nc.tensor.matmul(
                    hp[:, :],
                    w1_sb[:, cc * MID + mc * P : cc * MID + (mc + 1) * P],
                    embT4_r[:, cc, :],
                    start=(cc == 0),
                    stop=(cc == NCC - 1),
                )
            gf = ACTF.Gelu if GELU_KIND == "gelu" else ACTF.Identity
            nc.scalar.activation(
                h1_sb[:, mc * BL : (mc + 1) * BL],
                hp[:, :],
                gf,
                bias=b1t_sb[:, mc : mc + 1],
            )
        for pc in range(NPC):
            pp = ps_mix.tile([P, BL], FP, tag="mix")
            for mc in range(NMC):
                nc.tensor.matmul(
                    pp[:, :],
                    w2_sb[:, mc * PROJ + pc * P : mc * PROJ + (pc + 1) * P],
                    h1_sb[:, mc * BL : (mc + 1) * BL],
                    start=(mc == 0),
                    stop=(mc == NMC - 1),
                )
            nc.vector.tensor_scalar_add(
                pred_sb[:, pc * BL : (pc + 1) * BL], pp[:, :], b2t_sb[:, pc : pc + 1]
            )
            nc.sync.dma_start(
                out_d[pc * P : (pc + 1) * P, :], pred_sb[:, pc * BL : (pc + 1) * BL]
            )

    nc.compile()
    return nc


_NC_CACHE = None


def kernel(**inputs) -> np.ndarray:
    global _NC_CACHE
    from concourse.bass_utils import run_bass_kernel_spmd

    in_maps = _host_prep(inputs)
    if _NC_CACHE is None:
        _NC_CACHE = build_program()
    nc = _NC_CACHE
    res = run_bass_kernel_spmd(nc, in_maps, core_ids=list(range(NCORES)))
    out = np.empty((B, PROJ), np.float32)
    for c in range(NCORES):
        out[c * BL : (c + 1) * BL] = res.results[c]["predT"].T
    return out


# revision 5
# speedup vs baseline: 2.3617x; 1.3292x over previous
"""AttentionalPooler Trainium2 kernel: 8-core data-parallel over batch.

Math restructuring (host side, exact algebra):
  - q = LN(queries)@Wq*scale is batch-independent -> precomputed on host, shipped
    transposed as qT[d, h*N+i] in bf16.
  - LN mean-subtraction folds into column-centered weight matrices:
      LN(x)@W = rstd * (x @ center(diag(g)W))   (center = subtract column means)
    Applied to Wkv (ctx LN) and W1 (post LN).
  - Softmax denominators S_h[i] come free from a ones-augmented v matmul
    (row 64 of each head's psum); reciprocal via the fast custom-DVE op; 1/S
    rows are expanded to [128, N] head-pair tiles by a K=8 selector matmul
    on the PE (e8 one-hot lhsT), then folded into the oin in-place multiply.
  - rstd of the ctx LN is applied via the Exp activation's per-partition scale
    (for k) and folded into the v psum->sbuf evacuation (for v).

v3 (PE density): everything matmul-shaped runs bf16 (fp32 streams at 4
cycles/row and drew enough power to DVFS-throttle the core).  PSUM pools are
merged into 3 groups (transpose/sim/emb | vo/rb/kv | wout) so the sim psum is
double-buffered -- the PE no longer stalls behind the Exp activations, which
matters because the Tensor engine p-state only reaches 2.4 GHz after ~3us of
gapless execution.  attn@v runs in 4 two-head waves consuming es tiles in
exp-completion order.  The emb matmuls interleave one-by-one with the next
ic's wout matmuls so their LDWEIGHTS hide behind 512-col streams.  MLP is
bf16 with all weights preloaded to SBUF.  Scalar engine runs only Exp + Sqrt
+ final Gelu (activation-table thrash costs 1.3us per function switch).
"""

import sys

sys.path.insert(0, "/opt/trn_rl_repo")

import numpy as np
import ml_dtypes

import concourse.bacc as bacc
import concourse.mybir as mybir
import concourse.tile as tile
from concourse.masks import make_identity

B, N, D = 32, 512, 1024
H, DH = 8, 64
INNER = H * DH  # 512
PROJ = 512
MID = (D + PROJ) // 2  # 768
EPS = 1e-5
NCORES = 8
BL = B // NCORES  # 4 batch items per core
FP = mybir.dt.float32
BF = mybir.dt.bfloat16
ACTF = mybir.ActivationFunctionType

P = 128
GELU_KIND = "gelu"  # "identity" for CoreSim (no Gelu in interpreter)
NJC = N // P  # 4 j-chunks
NCC = D // P  # 8 c-chunks
NIC = N // P  # 4 i-chunks
NINC = INNER // P  # 4 inner-chunks
NMC = MID // P  # 6 mid-chunks
NPC = PROJ // P  # 4 proj-chunks

BF_NP = ml_dtypes.bfloat16


def _host_prep(inputs):
    x = np.asarray(inputs["x"], np.float32)
    pos = np.asarray(inputs["pos_encoding"], np.float32)[0]  # [N, D]
    queries = np.asarray(inputs["queries"], np.float32)
    ln_q_g = np.asarray(inputs["ln_q_g"], np.float32)
    ln_ctx_g = np.asarray(inputs["ln_ctx_g"], np.float32)
    Wq = np.asarray(inputs["Wq"], np.float32)
    Wkv = np.asarray(inputs["Wkv"], np.float32)
    Wout = np.asarray(inputs["Wout"], np.float32)
    ln_post_g = np.asarray(inputs["ln_post_g"], np.float32)
    W1 = np.asarray(inputs["W1"], np.float32)
    b1 = np.asarray(inputs["b1"], np.float32)
    W2 = np.asarray(inputs["W2"], np.float32)
    b2 = np.asarray(inputs["b2"], np.float32)

    # Batch-independent query projection.
    qm = queries.mean(1, keepdims=True)
    qv = queries.var(1, keepdims=True)
    qn = (queries - qm) / np.sqrt(qv + EPS) * ln_q_g
    q = (qn @ Wq) * (DH ** -0.5)  # [N, INNER]
    # qT[d, h*N + i] = q[i, h*DH + d]
    qT = np.ascontiguousarray(
        q.reshape(N, H, DH).transpose(2, 1, 0).reshape(DH, H * N).astype(BF_NP)
    )

    Wg = ln_ctx_g[:, None] * Wkv
    wkv_c = np.ascontiguousarray((Wg - Wg.mean(0, keepdims=True)).astype(BF_NP))

    W1g = ln_post_g[:, None] * W1
    w1_c = np.ascontiguousarray(((W1g - W1g.mean(0, keepdims=True)) / N).astype(BF_NP))

    b1t = np.ascontiguousarray(b1.reshape(NMC, P).T)  # [128, 6]
    b2t = np.ascontiguousarray(b2.reshape(NPC, P).T)  # [128, 4]

    # e8[k, kc*128 + m] = 1 iff k == 2*kc + m//64 : selects 1/S rows for the
    # head-pair kc, upper/lower 64 partitions.
    e8 = np.zeros((8, NINC * P), np.float32)
    for kc in range(NINC):
        e8[2 * kc, kc * P : kc * P + DH] = 1.0
        e8[2 * kc + 1, kc * P + DH : (kc + 1) * P] = 1.0

    common = {
        "pos": pos,
        "qT": qT,
        "wkv": wkv_c,
        "wout": np.ascontiguousarray(Wout.astype(BF_NP)),
        "w1": np.ascontiguousarray(w1_c),
        "b1t": b1t,
        "w2": np.ascontiguousarray(W2.astype(BF_NP)),
        "b2t": b2t,
        "e8": e8,
    }
    in_maps = []
    for c in range(NCORES):
        m = dict(common)
        m["x"] = np.ascontiguousarray(x[c * BL : (c + 1) * BL].reshape(BL * N, D))
        in_maps.append(m)
    return in_maps


def build_program():
    nc = bacc.Bacc("TRN2", target_bir_lowering=False, debug=False)
    x_d = nc.dram_tensor("x", [BL * N, D], FP, kind="ExternalInput")
    pos_d = nc.dram_tensor("pos", [N, D], FP, kind="ExternalInput")
    qT_d = nc.dram_tensor("qT", [DH, H * N], BF, kind="ExternalInput")
    wkv_d = nc.dram_tensor("wkv", [D, 2 * DH], BF, kind="ExternalInput")
    wout_d = nc.dram_tensor("wout", [INNER, D], BF, kind="ExternalInput")
    w1_d = nc.dram_tensor("w1", [D, MID], BF, kind="ExternalInput")
    b1t_d = nc.dram_tensor("b1t", [P, NMC], FP, kind="ExternalInput")
    w2_d = nc.dram_tensor("w2", [MID, PROJ], BF, kind="ExternalInput")
    b2t_d = nc.dram_tensor("b2t", [P, NPC], FP, kind="ExternalInput")
    e8_d = nc.dram_tensor("e8", [8, NINC * P], FP, kind="ExternalInput")
    out_d = nc.dram_tensor("predT", [PROJ, BL], FP, kind="ExternalOutput")

    from contextlib import ExitStack

    with tile.TileContext(nc) as tc, ExitStack() as ctx:
        pool = lambda name, bufs, **kw: ctx.enter_context(
            tc.tile_pool(name=name, bufs=bufs, **kw)
        )
        consts = pool("consts", 1)
        xraw_p = pool("xraw", 2)
        xnb_p = pool("xnb", 2)
        xT_p = pool("xT", 2)
        kv_p = pool("kv", 2)
        es_p = pool("es", 4)
        oin_p = pool("oin", 2)
        ost_p = pool("ost", 2)
        srow_p = pool("srow", 3)
        s8_p = pool("s8p", 2)
        onat_p = pool("onat", 1)
        small_p = pool("small", 2)
        # PSUM: 8 banks total.  a={transpose pairs, sim head-pairs, embps}
        # (4KB slot x2 = 4 banks), vo={vo, rb, kvps} (2KB x2 = 2 banks),
        # wout={wp} (4KB x1 = 2 banks).
        ps_a = pool("ps_a", 2, space="PSUM")
        ps_vo = pool("ps_vo", 2, space="PSUM")
        ps_wout = pool("ps_wout", 1, space="PSUM")

        identb = consts.tile([P, P], BF)
        make_identity(nc, identb)
        eps_sb = consts.tile([P, 1], FP)
        nc.vector.memset(eps_sb[:, :], EPS)

        pos_sb = consts.tile([P, NJC * D], FP)
        for jc in range(NJC):
            nc.sync.dma_start(
                pos_sb[:, jc * D : (jc + 1) * D], pos_d[jc * P : (jc + 1) * P, :]
            )
        wkv_sb = consts.tile([P, NCC * 2 * DH], BF)
        for cc in range(NCC):
            nc.sync.dma_start(
                wkv_sb[:, cc * 128 : (cc + 1) * 128],
                wkv_d[cc * P : (cc + 1) * P, :],
            )
        wout_sb = consts.tile([P, NINC * D], BF)
        for kc in range(NINC):
            nc.sync.dma_start(
                wout_sb[:, kc * D : (kc + 1) * D], wout_d[kc * P : (kc + 1) * P, :]
            )
        qT_sb = consts.tile([DH, H * N], BF)
        nc.sync.dma_start(qT_sb[:, :], qT_d[:, :])
        e8_sb = consts.tile([8, NINC * P], FP)
        nc.sync.dma_start(e8_sb[:, :], e8_d[:, :])
        b1t_sb = consts.tile([P, NMC], FP)
        nc.sync.dma_start(b1t_sb[:, :], b1t_d[:, :])
        b2t_sb = consts.tile([P, NPC], FP)
        nc.sync.dma_start(b2t_sb[:, :], b2t_d[:, :])
        w1_sb = consts.tile([P, NCC * MID], BF)
        for cc in range(NCC):
            nc.sync.dma_start(
                w1_sb[:, cc * MID : (cc + 1) * MID], w1_d[cc * P : (cc + 1) * P, :]
            )
        w2_sb = consts.tile([P, NMC * PROJ], BF)
        for mc in range(NMC):
            nc.sync.dma_start(
                w2_sb[:, mc * PROJ : (mc + 1) * PROJ],
                w2_d[mc * P : (mc + 1) * P, :],
            )
        embT4 = consts.tile([P, BL * NCC], FP)  # col = b*NCC + cc
        h1_sb = consts.tile([P, NMC * BL], BF)
        pred_sb = consts.tile([P, NPC * BL], FP)

        for b in range(BL):
            # ---- load x, add pos -> bf16 ----
            xr = xraw_p.tile([P, NJC * D], FP, tag="xr")
            for jc in range(NJC):
                nc.sync.dma_start(
                    xr[:, jc * D : (jc + 1) * D],
                    x_d[b * N + jc * P : b * N + (jc + 1) * P, :],
                )
            xnb = xnb_p.tile([P, NJC * D], BF, tag="xnb")
            for jc in range(NJC):
                s = slice(jc * D, (jc + 1) * D)
                nc.vector.tensor_add(xnb[:, s], xr[:, s], pos_sb[:, s])

            # ---- ctx LN rstd (stats on bf16; var error averages out) ----
            rstd = small_p.tile([P, NJC], FP, tag="rstd")
            sq = small_p.tile([P, NJC], FP, tag="sq")
            for jc in range(NJC):
                st = small_p.tile([P, 2, 6], FP, tag="bnst")
                for g in range(2):
                    nc.vector.bn_stats(
                        st[:, g, :],
                        xnb[:, jc * D + g * 512 : jc * D + (g + 1) * 512],
                    )
                ag = small_p.tile([P, 2], FP, tag="bnag")
                nc.vector.bn_aggr(ag[:, :], st[:, :, :])
                nc.scalar.activation(
                    sq[:, jc : jc + 1], ag[:, 1:2], ACTF.Sqrt, bias=eps_sb[:, :]
                )
            nc.vector.reciprocal(rstd[:, :], sq[:, :])

            # ---- transpose xnb -> xT [c-part, j-free] bf16; kv matmul ----
            kvps = ps_vo.tile([P, N], FP, tag="vo")
            for chalf in range(2):
                xT = xT_p.tile([P, 4 * N], BF, tag="xT")
                for ccp in range(2):  # pairs of c-chunks share one psum tile
                    pt = ps_a.tile([P, 2 * N], BF, tag="a")
                    for cci in range(2):
                        cc = chalf * 4 + ccp * 2 + cci
                        for jc in range(NJC):
                            nc.tensor.transpose(
                                pt[:, cci * N + jc * P : cci * N + (jc + 1) * P],
                                xnb[:, jc * D + cc * P : jc * D + (cc + 1) * P],
                                identb[:, :],
                            )
                    nc.vector.tensor_copy(
                        xT[:, ccp * 2 * N : (ccp + 1) * 2 * N], pt[:, :]
                    )
                for cc4 in range(4):
                    cc = chalf * 4 + cc4
                    nc.tensor.matmul(
                        kvps[:, :],
                        wkv_sb[:, cc * 128 : (cc + 1) * 128],
                        xT[:, cc4 * N : (cc4 + 1) * N],
                        start=(cc == 0),
                        stop=(cc == NCC - 1),
                    )
            kvT = kv_p.tile([P, N], BF, tag="kvT")
            nc.vector.tensor_copy(kvT[:, :], kvps[:, :])

            # ---- v natural [j-part, d-free] bf16, scaled by rstd[j] ----
            v_nat = kv_p.tile([P, NJC * (DH + 1)], BF, tag="vnat")
            for jc in range(NJC):
                vt = ps_a.tile([P, DH], BF, tag="a")
                nc.tensor.transpose(
                    vt[:, :],
                    kvT[DH:, jc * P : (jc + 1) * P],
                    identb[DH:P, DH:P],
                )
                nc.vector.tensor_scalar_mul(
                    v_nat[:, jc * 65 : jc * 65 + DH],
                    vt[:, :],
                    rstd[:, jc : jc + 1],
                )
                nc.vector.memset(v_nat[:, jc * 65 + DH : (jc + 1) * 65], 1.0)

            # ---- sim + exp per (jc, head-pair); psum double-buffered ----
            es_tiles = []
            for jc in range(NJC):
                es_t = es_p.tile([P, H * N], BF, tag="es")
                es_tiles.append(es_t)
                for hq in range(4):
                    sm = ps_a.tile([P, 2 * N], FP, tag="a")
                    for hh in range(2):
                        h = hq * 2 + hh
                        nc.tensor.matmul(
                            sm[:, hh * N : (hh + 1) * N],
                            kvT[0:DH, jc * P : (jc + 1) * P],
                            qT_sb[:, h * N : (h + 1) * N],
                            start=True,
                            stop=True,
                        )
                    nc.scalar.activation(
                        es_t[:, hq * 2 * N : (hq + 1) * 2 * N],
                        sm[:, :],
                        ACTF.Exp,
                        scale=rstd[:, jc : jc + 1],
                    )

            # ---- attn @ v in 2-head waves (es consumed in exp order) ----
            oin = oin_p.tile([P, NINC * N], BF, tag="oin")
            s8 = s8_p.tile([8, N], FP, tag="s8")
            for w in range(4):
                vo0 = ps_vo.tile([DH + 1, N], FP, tag="vo")
                vo1 = ps_vo.tile([DH + 1, N], FP, tag="vo")
                for jc in range(NJC):
                    for hh, vo in ((0, vo0), (1, vo1)):
                        h = 2 * w + hh
                        nc.tensor.matmul(
                            vo[:, :],
                            v_nat[:, jc * 65 : (jc + 1) * 65],
                            es_tiles[jc][:, h * N : (h + 1) * N],
                            start=(jc == 0),
                            stop=(jc == NJC - 1),
                        )
                for hh, vo in ((0, vo0), (1, vo1)):
                    h = 2 * w + hh
                    srow = srow_p.tile([P, N], FP, tag="srow")
                    nc.vector.tensor_copy(srow[DH : DH + 1, :], vo[DH : DH + 1, :])
                    nc.gpsimd.dma_start(s8[h : h + 1, :], srow[DH : DH + 1, :])
                    if hh == 0:
                        nc.vector.tensor_copy(
                            oin[0:DH, w * N : (w + 1) * N], vo[0:DH, :]
                        )
                    else:
                        ost = ost_p.tile([DH, N], BF, tag="ost")
                        nc.vector.tensor_copy(ost[:, :], vo[0:DH, :])
                        nc.gpsimd.dma_start(
                            oin[DH:P, w * N : (w + 1) * N], ost[:, :]
                        )
            s8r = s8_p.tile([8, N], FP, tag="s8r")
            nc.vector.reciprocal_approx_fast(s8r[:, :], s8[:, :])
            for kc in range(NINC):
                rb = ps_vo.tile([P, N], FP, tag="vo")
                nc.tensor.matmul(
                    rb[:, :],
                    e8_sb[:, kc * P : (kc + 1) * P],
                    s8r[:, :],
                    start=True,
                    stop=True,
                )
                nc.vector.tensor_mul(
                    oin[:, kc * N : (kc + 1) * N],
                    oin[:, kc * N : (kc + 1) * N],
                    rb[:, :],
                )

            # ---- Wout -> o natural; post-LN; emb (interleaved 1:1) ----
            onat = onat_p.tile([P, NIC * D], BF, tag="onat")
            rstdo = small_p.tile([P, NIC], FP, tag="rstdo")
            rstdob = small_p.tile([P, NIC], BF, tag="rstdob")
            embps = ps_a.tile([P, NIC, NCC], FP, tag="a")

            def emb_mms(ic):
                for cc in range(NCC):
                    yield lambda ic=ic, cc=cc: nc.tensor.matmul(
                        embps[:, ic, cc : cc + 1],
                        onat[:, ic * D + cc * P : ic * D + (cc + 1) * P],
                        rstdob[:, ic : ic + 1],
                        start=True,
                        stop=True,
                    )

            for ic in range(NIC):
                wp = ps_wout.tile([P, D], FP, tag="wout")
                prev = emb_mms(ic - 1) if ic > 0 else iter(())
                for kc in range(NINC):
                    for half in range(2):
                        nc.tensor.matmul(
                            wp[:, half * 512 : (half + 1) * 512],
                            oin[:, kc * N + ic * P : kc * N + (ic + 1) * P],
                            wout_sb[:, kc * D + half * 512 : kc * D + half * 512 + 512],
                            start=(kc == 0),
                            stop=(kc == NINC - 1),
                        )
                        nxt = next(prev, None)
                        if nxt is not None:
                            nxt()
                st2 = small_p.tile([P, 2, 6], FP, tag="bnst2")
                for g in range(2):
                    nc.vector.bn_stats(st2[:, g, :], wp[:, g * 512 : (g + 1) * 512])
                ag2 = small_p.tile([P, 2], FP, tag="bnag2")
                nc.vector.bn_aggr(ag2[:, :], st2[:, :, :])
                sq2 = small_p.tile([P, 1], FP, tag="sq2")
                nc.scalar.activation(
                    sq2[:, :], ag2[:, 1:2], ACTF.Sqrt, bias=eps_sb[:, :]
                )
                nc.vector.reciprocal(rstdo[:, ic : ic + 1], sq2[:, :])
                nc.gpsimd.tensor_copy(rstdob[:, ic : ic + 1], rstdo[:, ic : ic + 1])
                nc.vector.tensor_copy(onat[:, ic * D : (ic + 1) * D], wp[:, :])
            for mm in emb_mms(3):
                mm()
            ebt = embT4[:, b * NCC : (b + 1) * NCC]
            nc.vector.tensor_copy(ebt, embps[:, 0, :])
            for icp in range(1, NIC):
                nc.vector.tensor_add(ebt, ebt, embps[:, icp, :])

        # ---- MLP (bf16, weights preloaded in SBUF) ----
        embT4b = consts.tile([P, BL * NCC], BF)
        nc.vector.tensor_copy(embT4b[:, :], embT4[:, :])
        embT4_r = embT4b.rearrange("p (b c) -> p c b", c=NCC)
        for mc in range(NMC):
            hp = ps_a.tile([P, BL], FP, tag="a")
            for cc in range(NCC):
                nc.tensor.matmul(
                    hp[:, :],
                    w1_sb[:, cc * MID + mc * P : cc * MID + (mc + 1) * P],
                    embT4_r[:, cc, :],
                    start=(cc == 0),
                    stop=(cc == NCC - 1),
                )
            gf = ACTF.Gelu if GELU_KIND == "gelu" else ACTF.Identity
            nc.scalar.activation(
                h1_sb[:, mc * BL : (mc + 1) * BL],
                hp[:, :],
                gf,
                bias=b1t_sb[:, mc : mc + 1],
            )
        for pc in range(NPC):
            pp = ps_a.tile([P, BL], FP, tag="a")
            for mc in range(NMC):
                nc.tensor.matmul(
                    pp[:, :],
                    w2_sb[:, mc * PROJ + pc * P : mc * PROJ + (pc + 1) * P],
                    h1_sb[:, mc * BL : (mc + 1) * BL],
                    start=(mc == 0),
                    stop=(mc == NMC - 1),
                )
            nc.vector.tensor_scalar_add(
                pred_sb[:, pc * BL : (pc + 1) * BL], pp[:, :], b2t_sb[:, pc : pc + 1]
            )
            nc.sync.dma_start(
                out_d[pc * P : (pc + 1) * P, :], pred_sb[:, pc * BL : (pc + 1) * BL]
            )

    nc.compile()
    return nc


_NC_CACHE = None


def kernel(**inputs) -> np.ndarray:
    global _NC_CACHE
    from concourse.bass_utils import run_bass_kernel_spmd

    in_maps = _host_prep(inputs)
    if _NC_CACHE is None:
        _NC_CACHE = build_program()
    nc = _NC_CACHE
    res = run_bass_kernel_spmd(nc, in_maps, core_ids=list(range(NCORES)))
    out = np.empty((B, PROJ), np.float32)
    for c in range(NCORES):
        out[c * BL : (c + 1) * BL] = res.results[c]["predT"].T
    return out


# revision 6
# speedup vs baseline: 2.6723x; 1.1315x over previous
"""AttentionalPooler Trainium2 kernel: 8-core data-parallel over batch.

Math restructuring (host side, exact algebra):
  - q = LN(queries)@Wq*scale is batch-independent -> precomputed on host, shipped
    transposed as qT[d, h*N+i] in bf16.
  - LN mean-subtraction folds into column-centered weight matrices:
      LN(x)@W = rstd * (x @ center(diag(g)W))   (center = subtract column means)
    Applied to Wkv (ctx LN) and W1 (post LN).
  - Softmax denominators S_h[i] come free from a ones-augmented v matmul
    (row 64 of each head's psum); they ride the head-pair evacuation into SBUF,
    are gathered by tiny DMAs, inverted with the fast custom-DVE reciprocal,
    expanded to [128, N] head-pair tiles by a K=8 one-hot selector matmul on
    the PE, and folded into an in-place multiply.
  - rstd of the ctx LN is applied via the Exp activation's per-partition scale
    (for k) and folded into the v psum->sbuf evacuation (for v).

v4 (PE density via software pipelining): the Tensor engine p-state only
reaches 2.4 GHz after ~3us of gapless execution, so the whole kernel is a
3-stage pipeline keeping the PE queue dense: while the Scalar engine chews
batch b's 16 Exp activations (~19us, the pacer), the PE runs batch b-1's
Wout matmuls; the post-LN sqrts of b-1 queue on Scalar AFTER b's exps (no
mid-phase activation-table thrash); the emb matmuls of b-2 interleave 1:1
into b's transposes so their LDWEIGHTS hide behind longer streams.  All
matmul traffic is bf16 (fp32 streams at 4 cycles/row and drew enough power
to DVFS-throttle the core).  Weight preloads ride the GpSimd DMA queue so
batch 0's x loads aren't stuck behind 5 MB of weights.
"""

import sys

sys.path.insert(0, "/opt/trn_rl_repo")

import numpy as np
import ml_dtypes

import concourse.bacc as bacc
import concourse.mybir as mybir
import concourse.tile as tile
from concourse.masks import make_identity

B, N, D = 32, 512, 1024
H, DH = 8, 64
INNER = H * DH  # 512
PROJ = 512
MID = (D + PROJ) // 2  # 768
EPS = 1e-5
NCORES = 8
BL = B // NCORES  # 4 batch items per core
FP = mybir.dt.float32
BF = mybir.dt.bfloat16
ACTF = mybir.ActivationFunctionType

P = 128
GELU_KIND = "gelu"  # "identity" for CoreSim (no Gelu in interpreter)
NJC = N // P  # 4 j-chunks
NCC = D // P  # 8 c-chunks
NIC = N // P  # 4 i-chunks
NINC = INNER // P  # 4 inner-chunks
NMC = MID // P  # 6 mid-chunks
NPC = PROJ // P  # 4 proj-chunks

BF_NP = ml_dtypes.bfloat16


def _host_prep(inputs):
    x = np.asarray(inputs["x"], np.float32)
    pos = np.asarray(inputs["pos_encoding"], np.float32)[0]  # [N, D]
    queries = np.asarray(inputs["queries"], np.float32)
    ln_q_g = np.asarray(inputs["ln_q_g"], np.float32)
    ln_ctx_g = np.asarray(inputs["ln_ctx_g"], np.float32)
    Wq = np.asarray(inputs["Wq"], np.float32)
    Wkv = np.asarray(inputs["Wkv"], np.float32)
    Wout = np.asarray(inputs["Wout"], np.float32)
    ln_post_g = np.asarray(inputs["ln_post_g"], np.float32)
    W1 = np.asarray(inputs["W1"], np.float32)
    b1 = np.asarray(inputs["b1"], np.float32)
    W2 = np.asarray(inputs["W2"], np.float32)
    b2 = np.asarray(inputs["b2"], np.float32)

    # Batch-independent query projection.
    qm = queries.mean(1, keepdims=True)
    qv = queries.var(1, keepdims=True)
    qn = (queries - qm) / np.sqrt(qv + EPS) * ln_q_g
    q = (qn @ Wq) * (DH ** -0.5)  # [N, INNER]
    # qT[d, h*N + i] = q[i, h*DH + d]
    qT = np.ascontiguousarray(
        q.reshape(N, H, DH).transpose(2, 1, 0).reshape(DH, H * N).astype(BF_NP)
    )

    Wg = ln_ctx_g[:, None] * Wkv
    wkv_c = np.ascontiguousarray((Wg - Wg.mean(0, keepdims=True)).astype(BF_NP))

    W1g = ln_post_g[:, None] * W1
    w1_c = np.ascontiguousarray(((W1g - W1g.mean(0, keepdims=True)) / N).astype(BF_NP))

    b1t = np.ascontiguousarray(b1.reshape(NMC, P).T)  # [128, 6]
    b2t = np.ascontiguousarray(b2.reshape(NPC, P).T)  # [128, 4]

    # e8[k, kc*128 + m] = 1 iff k == 2*kc + m//64 : selects 1/S rows for the
    # head-pair kc, upper/lower 64 partitions.
    e8 = np.zeros((8, NINC * P), np.float32)
    for kc in range(NINC):
        e8[2 * kc, kc * P : kc * P + DH] = 1.0
        e8[2 * kc + 1, kc * P + DH : (kc + 1) * P] = 1.0

    common = {
        "pos": pos,
        "qT": qT,
        "wkv": wkv_c,
        "wout": np.ascontiguousarray(Wout.astype(BF_NP)),
        "w1": np.ascontiguousarray(w1_c),
        "b1t": b1t,
        "w2": np.ascontiguousarray(W2.astype(BF_NP)),
        "b2t": b2t,
        "e8": np.ascontiguousarray(e8.astype(BF_NP)),
    }
    in_maps = []
    for c in range(NCORES):
        m = dict(common)
        m["x"] = np.ascontiguousarray(x[c * BL : (c + 1) * BL].reshape(BL * N, D))
        in_maps.append(m)
    return in_maps


def build_program():
    nc = bacc.Bacc("TRN2", target_bir_lowering=False, debug=False)
    x_d = nc.dram_tensor("x", [BL * N, D], FP, kind="ExternalInput")
    pos_d = nc.dram_tensor("pos", [N, D], FP, kind="ExternalInput")
    qT_d = nc.dram_tensor("qT", [DH, H * N], BF, kind="ExternalInput")
    wkv_d = nc.dram_tensor("wkv", [D, 2 * DH], BF, kind="ExternalInput")
    wout_d = nc.dram_tensor("wout", [INNER, D], BF, kind="ExternalInput")
    w1_d = nc.dram_tensor("w1", [D, MID], BF, kind="ExternalInput")
    b1t_d = nc.dram_tensor("b1t", [P, NMC], FP, kind="ExternalInput")
    w2_d = nc.dram_tensor("w2", [MID, PROJ], BF, kind="ExternalInput")
    b2t_d = nc.dram_tensor("b2t", [P, NPC], FP, kind="ExternalInput")
    e8_d = nc.dram_tensor("e8", [8, NINC * P], BF, kind="ExternalInput")
    out_d = nc.dram_tensor("predT", [PROJ, BL], FP, kind="ExternalOutput")

    from contextlib import ExitStack

    with tile.TileContext(nc) as tc, ExitStack() as ctx:
        pool = lambda name, bufs, **kw: ctx.enter_context(
            tc.tile_pool(name=name, bufs=bufs, **kw)
        )
        consts = pool("consts", 1)
        xraw_p = pool("xraw", 2)
        xnb_p = pool("xnb", 2)
        xT_p = pool("xT", 2)
        kv_p = pool("kv", 2)
        es_p = pool("es", 4)
        ov_p = pool("ov", 8)
        ost_p = pool("ost", 2)
        s8_p = pool("s8p", 2)
        onat_p = pool("onat", 1)
        small_p = pool("small", 2)
        # PSUM: 8 banks.  a={transpose pairs, sim head-pairs, MLP} (4KB x2 =
        # 4 banks), vo={kvps, vo, rb, embps} (2KB x2 = 2 banks),
        # wout={wp} (4KB x1 = 2 banks).
        ps_a = pool("ps_a", 2, space="PSUM")
        ps_vo = pool("ps_vo", 2, space="PSUM")
        ps_wout = pool("ps_wout", 1, space="PSUM")

        identb = consts.tile([P, P], BF)
        make_identity(nc, identb)
        eps_sb = consts.tile([P, 1], FP)
        nc.vector.memset(eps_sb[:, :], EPS)

        # Front-of-queue DMAs (needed by batch 0 immediately) on sync;
        # bulk weights on the gpsimd queue.
        pos_sb = consts.tile([P, NJC * D], FP)
        for jc in range(NJC):
            nc.sync.dma_start(
                pos_sb[:, jc * D : (jc + 1) * D], pos_d[jc * P : (jc + 1) * P, :]
            )
        wkv_sb = consts.tile([P, NCC * 2 * DH], BF)
        for cc in range(NCC):
            nc.sync.dma_start(
                wkv_sb[:, cc * 128 : (cc + 1) * 128],
                wkv_d[cc * P : (cc + 1) * P, :],
            )
        qT_sb = consts.tile([DH, H * N], BF)
        nc.sync.dma_start(qT_sb[:, :], qT_d[:, :])
        e8_sb = consts.tile([8, NINC * P], BF)
        nc.sync.dma_start(e8_sb[:, :], e8_d[:, :])
        b1t_sb = consts.tile([P, NMC], FP)
        nc.sync.dma_start(b1t_sb[:, :], b1t_d[:, :])
        b2t_sb = consts.tile([P, NPC], FP)
        nc.sync.dma_start(b2t_sb[:, :], b2t_d[:, :])
        wout_sb = consts.tile([P, NINC * D], BF)
        for kc in range(NINC):
            nc.gpsimd.dma_start(
                wout_sb[:, kc * D : (kc + 1) * D], wout_d[kc * P : (kc + 1) * P, :]
            )
        w1_sb = consts.tile([P, NCC * MID], BF)
        for cc in range(NCC):
            nc.gpsimd.dma_start(
                w1_sb[:, cc * MID : (cc + 1) * MID], w1_d[cc * P : (cc + 1) * P, :]
            )
        w2_sb = consts.tile([P, NMC * PROJ], BF)
        for mc in range(NMC):
            nc.gpsimd.dma_start(
                w2_sb[:, mc * PROJ : (mc + 1) * PROJ],
                w2_d[mc * P : (mc + 1) * P, :],
            )
        embT4 = consts.tile([P, BL * NCC], FP)  # col = b*NCC + cc
        h1_sb = consts.tile([P, NMC * BL], BF)
        pred_sb = consts.tile([P, NPC * BL], FP)

        # Per-b deferred state.
        ovs = {}  # b -> [4 ov tiles]
        onats = {}
        rstdobs = {}
        embpss = {}

        def emit_E1(bb):
            """Wout matmuls + post-LN stats + onat evac for batch bb.
            No scalar-engine ops (those are E2)."""
            onat = onat_p.tile([P, NIC * D], BF, tag="onat", name=f"onat{bb}")
            onats[bb] = onat
            ag2 = small_p.tile([P, NIC, 2], FP, tag="bnag2", name=f"ag2_{bb}")
            for ic in range(NIC):
                wp = ps_wout.tile([P, D], FP, tag="wout", name=f"wp{bb}_{ic}")
                for kc in range(NINC):
                    for half in range(2):
                        nc.tensor.matmul(
                            wp[:, half * 512 : (half + 1) * 512],
                            ovs[bb][kc][:, ic * P : (ic + 1) * P],
                            wout_sb[:, kc * D + half * 512 : kc * D + half * 512 + 512],
                            start=(kc == 0),
                            stop=(kc == NINC - 1),
                        )
                st2 = small_p.tile([P, 2, 6], FP, tag="bnst2", name=f"st2_{bb}_{ic}")
                for g in range(2):
                    nc.vector.bn_stats(st2[:, g, :], wp[:, g * 512 : (g + 1) * 512])
                nc.vector.bn_aggr(ag2[:, ic, :], st2[:, :, :])
                nc.vector.tensor_copy(onat[:, ic * D : (ic + 1) * D], wp[:, :])
            return ag2

        def emit_E2(bb, ag2):
            """Scalar sqrts (queued after batch bb+1's exps) + recip + cast."""
            sq2 = small_p.tile([P, NIC], FP, tag="sq2", name=f"sq2_{bb}")
            rstdo = small_p.tile([P, NIC], FP, tag="rstdo", name=f"rstdo{bb}")
            rstdob = small_p.tile([P, NIC], BF, tag="rstdob", name=f"rstdob{bb}")
            rstdobs[bb] = rstdob
            for ic in range(NIC):
                nc.scalar.activation(
                    sq2[:, ic : ic + 1], ag2[:, ic, 1:2], ACTF.Sqrt, bias=eps_sb[:, :]
                )
            nc.vector.reciprocal(rstdo[:, :], sq2[:, :])
            nc.gpsimd.tensor_copy(rstdob[:, :], rstdo[:, :])

        def emb_mms(bb):
            """32 one-col emb matmuls for batch bb, as closures."""
            embps = ps_vo.tile([P, NIC, NCC], FP, tag="vo", name=f"embps{bb}")
            embpss[bb] = embps
            onat, rstdob = onats[bb], rstdobs[bb]
            for ic in range(NIC):
                for cc in range(NCC):
                    yield lambda ic=ic, cc=cc: nc.tensor.matmul(
                        embps[:, ic, cc : cc + 1],
                        onat[:, ic * D + cc * P : ic * D + (cc + 1) * P],
                        rstdob[:, ic : ic + 1],
                        start=True,
                        stop=True,
                    )

        def emit_embT4(bb):
            embps = embpss.pop(bb)
            ebt = embT4[:, bb * NCC : (bb + 1) * NCC]
            nc.vector.tensor_copy(ebt, embps[:, 0, :])
            for icp in range(1, NIC):
                nc.vector.tensor_add(ebt, ebt, embps[:, icp, :])

        for b in range(BL):
            # ---- A: load x, add pos -> bf16, ctx LN rstd ----
            xr = xraw_p.tile([P, NJC * D], FP, tag="xr")
            for jc in range(NJC):
                nc.sync.dma_start(
                    xr[:, jc * D : (jc + 1) * D],
                    x_d[b * N + jc * P : b * N + (jc + 1) * P, :],
                )
            xnb = xnb_p.tile([P, NJC * D], BF, tag="xnb")
            for jc in range(NJC):
                s = slice(jc * D, (jc + 1) * D)
                nc.vector.tensor_add(xnb[:, s], xr[:, s], pos_sb[:, s])
            rstd = small_p.tile([P, NJC], FP, tag="rstd")
            sq = small_p.tile([P, NJC], FP, tag="sq")
            for jc in range(NJC):
                st = small_p.tile([P, 2, 6], FP, tag="bnst")
                for g in range(2):
                    nc.vector.bn_stats(
                        st[:, g, :],
                        xnb[:, jc * D + g * 512 : jc * D + (g + 1) * 512],
                    )
                ag = small_p.tile([P, 2], FP, tag="bnag")
                nc.vector.bn_aggr(ag[:, :], st[:, :, :])
                nc.scalar.activation(
                    sq[:, jc : jc + 1], ag[:, 1:2], ACTF.Sqrt, bias=eps_sb[:, :]
                )
            nc.vector.reciprocal(rstd[:, :], sq[:, :])

            # ---- B: transpose -> xT bf16, kv matmul; emb(b-2) interleaved --
            emb_it = emb_mms(b - 2) if b >= 2 else iter(())
            kvps = ps_vo.tile([P, N], FP, tag="vo", name=f"kvps{b}")
            for chalf in range(2):
                xT = xT_p.tile([P, 4 * N], BF, tag="xT")
                for ccp in range(2):
                    pt = ps_a.tile([P, 2 * N], BF, tag="a")
                    for cci in range(2):
                        cc = chalf * 4 + ccp * 2 + cci
                        for jc in range(NJC):
                            nc.tensor.transpose(
                                pt[:, cci * N + jc * P : cci * N + (jc + 1) * P],
                                xnb[:, jc * D + cc * P : jc * D + (cc + 1) * P],
                                identb[:, :],
                            )
                            mm = next(emb_it, None)
                            if mm is not None:
                                mm()
                    nc.vector.tensor_copy(
                        xT[:, ccp * 2 * N : (ccp + 1) * 2 * N], pt[:, :]
                    )
                for cc4 in range(4):
                    cc = chalf * 4 + cc4
                    nc.tensor.matmul(
                        kvps[:, :],
                        wkv_sb[:, cc * 128 : (cc + 1) * 128],
                        xT[:, cc4 * N : (cc4 + 1) * N],
                        start=(cc == 0),
                        stop=(cc == NCC - 1),
                    )
            kvT = kv_p.tile([P, N], BF, tag="kvT")
            nc.vector.tensor_copy(kvT[:, :], kvps[:, :])
            if b >= 2:
                emit_embT4(b - 2)

            # ---- v natural [j-part, d-free] bf16, scaled by rstd[j] ----
            v_nat = kv_p.tile([P, NJC * (DH + 1)], BF, tag="vnat")
            for jc in range(NJC):
                vt = ps_a.tile([P, DH], BF, tag="a")
                nc.tensor.transpose(
                    vt[:, :],
                    kvT[DH:, jc * P : (jc + 1) * P],
                    identb[DH:P, DH:P],
                )
                nc.vector.tensor_scalar_mul(
                    v_nat[:, jc * 65 : jc * 65 + DH],
                    vt[:, :],
                    rstd[:, jc : jc + 1],
                )
                nc.vector.memset(v_nat[:, jc * 65 + DH : (jc + 1) * 65], 1.0)

            # ---- C: sim + exp per (jc, head-pair); psum double-buffered ----
            es_tiles = []
            for jc in range(NJC):
                es_t = es_p.tile([P, H * N], BF, tag="es")
                es_tiles.append(es_t)
                for hq in range(4):
                    sm = ps_a.tile([P, 2 * N], FP, tag="a")
                    for hh in range(2):
                        h = hq * 2 + hh
                        nc.tensor.matmul(
                            sm[:, hh * N : (hh + 1) * N],
                            kvT[0:DH, jc * P : (jc + 1) * P],
                            qT_sb[:, h * N : (h + 1) * N],
                            start=True,
                            stop=True,
                        )
                    nc.scalar.activation(
                        es_t[:, hq * 2 * N : (hq + 1) * 2 * N],
                        sm[:, :],
                        ACTF.Exp,
                        scale=rstd[:, jc : jc + 1],
                    )

            # ---- E(b-1): wout on PE while scalar drains b's exps ----
            if b >= 1:
                ag2 = emit_E1(b - 1)
                emit_E2(b - 1, ag2)

            # ---- D: attn @ v in 2-head waves; S rides the evacuation ----
            ov4 = []
            ovs[b] = ov4
            s8 = s8_p.tile([8, N], BF, tag="s8")
            for w in range(4):
                vo0 = ps_vo.tile([DH + 1, N], FP, tag="vo")
                vo1 = ps_vo.tile([DH + 1, N], FP, tag="vo")
                for jc in range(NJC):
                    for hh, vo in ((0, vo0), (1, vo1)):
                        h = 2 * w + hh
                        nc.tensor.matmul(
                            vo[:, :],
                            v_nat[:, jc * 65 : (jc + 1) * 65],
                            es_tiles[jc][:, h * N : (h + 1) * N],
                            start=(jc == 0),
                            stop=(jc == NJC - 1),
                        )
                ov = ov_p.tile([P, N], BF, tag="ov", name=f"ov{b}_{w}")
                ov4.append(ov)
                # Even head + its S row (partition 64) in one copy.
                nc.vector.tensor_copy(ov[0 : DH + 1, :], vo0[:, :])
                nc.gpsimd.dma_start(s8[2 * w : 2 * w + 1, :], ov[DH : DH + 1, :])
                ost = ost_p.tile([DH + 1, N], BF, tag="ost")
                nc.vector.tensor_copy(ost[:, :], vo1[:, :])
                nc.gpsimd.dma_start(
                    s8[2 * w + 1 : 2 * w + 2, :], ost[DH : DH + 1, :]
                )
                # Odd v overwrites the S row region (queued after the S DMAs).
                nc.gpsimd.dma_start(ov[DH:P, :], ost[0:DH, :])
            s8f = s8_p.tile([8, N], FP, tag="s8f")
            nc.vector.tensor_copy(s8f[:, :], s8[:, :])
            s8r = s8_p.tile([8, N], FP, tag="s8r")
            nc.vector.reciprocal_approx_fast(s8r[:, :], s8f[:, :])
            s8b = s8_p.tile([8, N], BF, tag="s8b")
            nc.gpsimd.tensor_copy(s8b[:, :], s8r[:, :])
            for w in range(4):
                rb = ps_vo.tile([P, N], FP, tag="vo")
                nc.tensor.matmul(
                    rb[:, :],
                    e8_sb[:, w * P : (w + 1) * P],
                    s8b[:, :],
                    start=True,
                    stop=True,
                )
                nc.vector.tensor_mul(ov4[w][:, :], ov4[w][:, :], rb[:, :])

        # ---- tail: E(3), emb(2), emb(3), MLP ----
        ag2 = emit_E1(BL - 1)
        emit_E2(BL - 1, ag2)
        for mm in emb_mms(BL - 2):
            mm()
        emit_embT4(BL - 2)
        for mm in emb_mms(BL - 1):
            mm()
        emit_embT4(BL - 1)

        embT4b = consts.tile([P, BL * NCC], BF)
        nc.vector.tensor_copy(embT4b[:, :], embT4[:, :])
        embT4_r = embT4b.rearrange("p (b c) -> p c b", c=NCC)
        for mc in range(NMC):
            hp = ps_a.tile([P, BL], FP, tag="a")
            for cc in range(NCC):
                nc.tensor.matmul(
                    hp[:, :],
                    w1_sb[:, cc * MID + mc * P : cc * MID + (mc + 1) * P],
                    embT4_r[:, cc, :],
                    start=(cc == 0),
                    stop=(cc == NCC - 1),
                )
            gf = ACTF.Gelu if GELU_KIND == "gelu" else ACTF.Identity
            nc.scalar.activation(
                h1_sb[:, mc * BL : (mc + 1) * BL],
                hp[:, :],
                gf,
                bias=b1t_sb[:, mc : mc + 1],
            )
        for pc in range(NPC):
            pp = ps_a.tile([P, BL], FP, tag="a")
            for mc in range(NMC):
                nc.tensor.matmul(
                    pp[:, :],
                    w2_sb[:, mc * PROJ + pc * P : mc * PROJ + (pc + 1) * P],
                    h1_sb[:, mc * BL : (mc + 1) * BL],
                    start=(mc == 0),
                    stop=(mc == NMC - 1),
                )
            nc.vector.tensor_scalar_add(
                pred_sb[:, pc * BL : (pc + 1) * BL], pp[:, :], b2t_sb[:, pc : pc + 1]
            )
            nc.sync.dma_start(
                out_d[pc * P : (pc + 1) * P, :], pred_sb[:, pc * BL : (pc + 1) * BL]
            )

    nc.compile()
    return nc


_NC_CACHE = None


def kernel(**inputs) -> np.ndarray:
    global _NC_CACHE
    from concourse.bass_utils import run_bass_kernel_spmd

    in_maps = _host_prep(inputs)
    if _NC_CACHE is None:
        _NC_CACHE = build_program()
    nc = _NC_CACHE
    res = run_bass_kernel_spmd(nc, in_maps, core_ids=list(range(NCORES)))
    out = np.empty((B, PROJ), np.float32)
    for c in range(NCORES):
        out[c * BL : (c + 1) * BL] = res.results[c]["predT"].T
    return out


# revision 13
# speedup vs baseline: 3.2752x; 1.2256x over previous
"""AttentionalPooler Trainium2 kernel: 8-core data-parallel over batch.

Math restructuring (host side, exact algebra):
  - q = LN(queries)@Wq*scale is batch-independent -> precomputed on host, shipped
    transposed as qT[d, h*N+i] in bf16.
  - LN mean-subtraction folds into column-centered weight matrices:
      LN(x)@W = rstd * (x @ center(diag(g)W))   (center = subtract column means)
    Applied to Wkv (ctx LN) and W1 (post LN).
  - Softmax denominators S_h[i] come free from a ones-augmented v matmul
    (row 64 of each head's psum); they ride the head-pair evacuation into SBUF,
    are gathered by tiny DMAs, inverted with the fast custom-DVE reciprocal,
    expanded to [128, N] head-pair tiles by a K=8 one-hot selector matmul on
    the PE, and folded into an in-place multiply.
  - rstd of the ctx LN is applied via the Exp activation's per-partition scale
    (for k) and folded into the v psum->sbuf evacuation (for v).

v4 (PE density via software pipelining): the Tensor engine p-state only
reaches 2.4 GHz after ~3us of gapless execution, so the whole kernel is a
3-stage pipeline keeping the PE queue dense: while the Scalar engine chews
batch b's 16 Exp activations (~19us, the pacer), the PE runs batch b-1's
Wout matmuls; the post-LN sqrts of b-1 queue on Scalar AFTER b's exps (no
mid-phase activation-table thrash); the emb matmuls of b-2 interleave 1:1
into b's transposes so their LDWEIGHTS hide behind longer streams.  All
matmul traffic is bf16 (fp32 streams at 4 cycles/row and drew enough power
to DVFS-throttle the core).  Weight preloads ride the GpSimd DMA queue so
batch 0's x loads aren't stuck behind 5 MB of weights.
"""

import sys

sys.path.insert(0, "/opt/trn_rl_repo")

import numpy as np
import ml_dtypes

import concourse.bacc as bacc
import concourse.mybir as mybir
import concourse.tile as tile
from concourse.masks import make_identity

B, N, D = 32, 512, 1024
H, DH = 8, 64
INNER = H * DH  # 512
PROJ = 512
MID = (D + PROJ) // 2  # 768
EPS = 1e-5
NCORES = 8
BL = B // NCORES  # 4 batch items per core
FP = mybir.dt.float32
BF = mybir.dt.bfloat16
ACTF = mybir.ActivationFunctionType

P = 128
GELU_KIND = "gelu"  # "identity" for CoreSim (no Gelu in interpreter)
NJC = N // P  # 4 j-chunks
NCC = D // P  # 8 c-chunks
NIC = N // P  # 4 i-chunks
NINC = INNER // P  # 4 inner-chunks
NMC = MID // P  # 6 mid-chunks
NPC = PROJ // P  # 4 proj-chunks

BF_NP = ml_dtypes.bfloat16


def _host_prep(inputs):
    x = np.asarray(inputs["x"], np.float32)
    pos = np.asarray(inputs["pos_encoding"], np.float32)[0]  # [N, D]
    queries = np.asarray(inputs["queries"], np.float32)
    ln_q_g = np.asarray(inputs["ln_q_g"], np.float32)
    ln_ctx_g = np.asarray(inputs["ln_ctx_g"], np.float32)
    Wq = np.asarray(inputs["Wq"], np.float32)
    Wkv = np.asarray(inputs["Wkv"], np.float32)
    Wout = np.asarray(inputs["Wout"], np.float32)
    ln_post_g = np.asarray(inputs["ln_post_g"], np.float32)
    W1 = np.asarray(inputs["W1"], np.float32)
    b1 = np.asarray(inputs["b1"], np.float32)
    W2 = np.asarray(inputs["W2"], np.float32)
    b2 = np.asarray(inputs["b2"], np.float32)

    # Batch-independent query projection.
    qm = queries.mean(1, keepdims=True)
    qv = queries.var(1, keepdims=True)
    qn = (queries - qm) / np.sqrt(qv + EPS) * ln_q_g
    q = (qn @ Wq) * (DH ** -0.5)  # [N, INNER]
    # qT[d, h*N + i] = q[i, h*DH + d]
    qT = np.ascontiguousarray(
        q.reshape(N, H, DH).transpose(2, 1, 0).reshape(DH, H * N).astype(BF_NP)
    )

    Wg = ln_ctx_g[:, None] * Wkv
    wkv_c = np.ascontiguousarray((Wg - Wg.mean(0, keepdims=True)).astype(BF_NP))

    W1g = ln_post_g[:, None] * W1
    w1_c = np.ascontiguousarray(((W1g - W1g.mean(0, keepdims=True)) / N).astype(BF_NP))

    b1t = np.ascontiguousarray(b1.reshape(NMC, P).T)  # [128, 6]
    b2t = np.ascontiguousarray(b2.reshape(NPC, P).T)  # [128, 4]

    # e8[k, kc*128 + m] = 1 iff k == 2*kc + m//64 : selects 1/S rows for the
    # head-pair kc, upper/lower 64 partitions.
    e8 = np.zeros((8, NINC * P), np.float32)
    for kc in range(NINC):
        e8[2 * kc, kc * P : kc * P + DH] = 1.0
        e8[2 * kc + 1, kc * P + DH : (kc + 1) * P] = 1.0

    common = {
        "pos": np.ascontiguousarray(pos.astype(BF_NP)),
        "qT": qT,
        "wkv": wkv_c,
        "wout": np.ascontiguousarray(Wout.astype(BF_NP)),
        "w1": np.ascontiguousarray(w1_c),
        "b1t": b1t,
        "w2": np.ascontiguousarray(W2.astype(BF_NP)),
        "b2t": b2t,
        "e8": e8,
    }
    in_maps = []
    for c in range(NCORES):
        m = dict(common)
        m["x"] = np.ascontiguousarray(
            x[c * BL : (c + 1) * BL].reshape(BL * N, D).astype(BF_NP)
        )
        in_maps.append(m)
    return in_maps


def build_program():
    nc = bacc.Bacc("TRN2", target_bir_lowering=False, debug=False)
    x_d = nc.dram_tensor("x", [BL * N, D], BF, kind="ExternalInput")
    pos_d = nc.dram_tensor("pos", [N, D], BF, kind="ExternalInput")
    qT_d = nc.dram_tensor("qT", [DH, H * N], BF, kind="ExternalInput")
    wkv_d = nc.dram_tensor("wkv", [D, 2 * DH], BF, kind="ExternalInput")
    wout_d = nc.dram_tensor("wout", [INNER, D], BF, kind="ExternalInput")
    w1_d = nc.dram_tensor("w1", [D, MID], BF, kind="ExternalInput")
    b1t_d = nc.dram_tensor("b1t", [P, NMC], FP, kind="ExternalInput")
    w2_d = nc.dram_tensor("w2", [MID, PROJ], BF, kind="ExternalInput")
    b2t_d = nc.dram_tensor("b2t", [P, NPC], FP, kind="ExternalInput")
    e8_d = nc.dram_tensor("e8", [8, NINC * P], FP, kind="ExternalInput")
    out_d = nc.dram_tensor("predT", [PROJ, BL], FP, kind="ExternalOutput")

    from contextlib import ExitStack

    with tile.TileContext(nc) as tc, ExitStack() as ctx:
        pool = lambda name, bufs, **kw: ctx.enter_context(
            tc.tile_pool(name=name, bufs=bufs, **kw)
        )
        consts = pool("consts", 1)
        xraw_p = pool("xraw", 2)
        xnb_p = pool("xnb", 2)
        xT_p = pool("xT", 2)
        kv_p = pool("kv", 2)
        es_p = pool("es", 4)
        ov_p = pool("ov", 8)
        ost_p = pool("ost", 2)
        s8_p = pool("s8p", 2)
        onat_p = pool("onat", 1)
        small_p = pool("small", 2)
        # PSUM: 8 banks.  a={transpose pairs, sim head-pairs, MLP} (4KB x2 =
        # 4 banks), vo={kvps, vo, rb, embps} (2KB x2 = 2 banks),
        # wout={wp} (4KB x1 = 2 banks).
        ps_a = pool("ps_a", 2, space="PSUM")
        ps_vo = pool("ps_vo", 2, space="PSUM")
        ps_wout = pool("ps_wout", 1, space="PSUM")

        identb = consts.tile([P, P], BF)
        make_identity(nc, identb)
        eps_sb = consts.tile([P, 1], FP)
        nc.vector.memset(eps_sb[:, :], EPS)

        # Front-of-queue DMAs (needed by batch 0 immediately) on sync;
        # bulk weights on the gpsimd queue.
        pos_sb = consts.tile([P, NJC * D], BF)
        for chalf in range(2):
            for jc in range(NJC):
                cs = slice(chalf * 512, (chalf + 1) * 512)
                nc.sync.dma_start(
                    pos_sb[:, jc * D + chalf * 512 : jc * D + (chalf + 1) * 512],
                    pos_d[jc * P : (jc + 1) * P, cs],
                )
            if chalf == 0:
                wkv_sb = consts.tile([P, NCC * 2 * DH], BF)
                for cc in range(NCC):
                    nc.sync.dma_start(
                        wkv_sb[:, cc * 128 : (cc + 1) * 128],
                        wkv_d[cc * P : (cc + 1) * P, :],
                    )
        qT_sb = consts.tile([DH, H * N], BF)
        nc.sync.dma_start(qT_sb[:, :], qT_d[:, :])
        e8_sb = consts.tile([8, NINC * P], FP)
        nc.sync.dma_start(e8_sb[:, :], e8_d[:, :])
        b1t_sb = consts.tile([P, NMC], FP)
        nc.sync.dma_start(b1t_sb[:, :], b1t_d[:, :])
        b2t_sb = consts.tile([P, NPC], FP)
        nc.sync.dma_start(b2t_sb[:, :], b2t_d[:, :])
        wout_sb = consts.tile([P, NINC * D], BF)
        for kc in range(NINC):
            nc.gpsimd.dma_start(
                wout_sb[:, kc * D : (kc + 1) * D], wout_d[kc * P : (kc + 1) * P, :]
            )
        w1_sb = consts.tile([P, NCC * MID], BF)
        for cc in range(NCC):
            nc.gpsimd.dma_start(
                w1_sb[:, cc * MID : (cc + 1) * MID], w1_d[cc * P : (cc + 1) * P, :]
            )
        w2_sb = consts.tile([P, NMC * PROJ], BF)
        for mc in range(NMC):
            nc.gpsimd.dma_start(
                w2_sb[:, mc * PROJ : (mc + 1) * PROJ],
                w2_d[mc * P : (mc + 1) * P, :],
            )
        embT4 = consts.tile([P, BL * NCC], FP)  # col = b*NCC + cc
        h1_sb = consts.tile([P, NMC * BL], BF)
        pred_sb = consts.tile([P, NPC * BL], FP)

        # Per-b deferred state.
        ovs = {}  # b -> [4 ov tiles]
        onats = {}
        rstdobs = {}
        embpss = {}

        def emit_E1(bb):
            """Wout matmuls + post-LN stats + onat evac for batch bb.
            No scalar-engine ops (those are E2)."""
            onat = onat_p.tile([P, NIC * D], BF, tag="onat", name=f"onat{bb}")
            onats[bb] = onat
            ag2 = small_p.tile([P, NIC, 2], FP, tag="bnag2", name=f"ag2_{bb}")
            for ic in range(NIC):
                wp = ps_wout.tile([P, D], FP, tag="wout", name=f"wp{bb}_{ic}")
                for kc in range(NINC):
                    for half in range(2):
                        nc.tensor.matmul(
                            wp[:, half * 512 : (half + 1) * 512],
                            ovs[bb][kc][:, ic * P : (ic + 1) * P],
                            wout_sb[:, kc * D + half * 512 : kc * D + half * 512 + 512],
                            start=(kc == 0),
                            stop=(kc == NINC - 1),
                        )
                st2 = small_p.tile([P, 2, 6], FP, tag="bnst2", name=f"st2_{bb}_{ic}")
                for g in range(2):
                    nc.vector.bn_stats(st2[:, g, :], wp[:, g * 512 : (g + 1) * 512])
                nc.vector.bn_aggr(ag2[:, ic, :], st2[:, :, :])
                nc.vector.tensor_copy(onat[:, ic * D : (ic + 1) * D], wp[:, :])
            return ag2

        def emit_E2(bb, ag2):
            """Scalar sqrts (queued after batch bb+1's exps) + recip + cast."""
            sq2 = small_p.tile([P, NIC], FP, tag="sq2", name=f"sq2_{bb}")
            rstdo = small_p.tile([P, NIC], FP, tag="rstdo", name=f"rstdo{bb}")
            rstdob = small_p.tile([P, NIC], BF, tag="rstdob", name=f"rstdob{bb}")
            rstdobs[bb] = rstdob
            for ic in range(NIC):
                nc.scalar.activation(
                    sq2[:, ic : ic + 1], ag2[:, ic, 1:2], ACTF.Sqrt, bias=eps_sb[:, :]
                )
            nc.vector.reciprocal(rstdo[:, :], sq2[:, :])
            nc.gpsimd.tensor_copy(rstdob[:, :], rstdo[:, :])

        def emb_mms(bb):
            """32 one-col emb matmuls for batch bb, as closures."""
            embps = ps_vo.tile([P, NIC, NCC], FP, tag="vo", name=f"embps{bb}")
            embpss[bb] = embps
            onat, rstdob = onats[bb], rstdobs[bb]
            for ic in range(NIC):
                for cc in range(NCC):
                    yield lambda ic=ic, cc=cc: nc.tensor.matmul(
                        embps[:, ic, cc : cc + 1],
                        onat[:, ic * D + cc * P : ic * D + (cc + 1) * P],
                        rstdob[:, ic : ic + 1],
                        start=True,
                        stop=True,
                    )

        def emit_embT4(bb):
            embps = embpss.pop(bb)
            ebt = embT4[:, bb * NCC : (bb + 1) * NCC]
            nc.vector.tensor_copy(ebt, embps[:, 0, :])
            for icp in range(1, NIC):
                nc.vector.tensor_add(ebt, ebt, embps[:, icp, :])

        for b in range(BL):
            # ---- A: load x (by column half), add pos -> bf16, ctx rstd ----
            xr = xraw_p.tile([P, NJC * D], BF, tag="xr")
            xnb = xnb_p.tile([P, NJC * D], BF, tag="xnb")
            st4 = small_p.tile([P, NJC, 2, 6], FP, tag="bnst")
            for chalf in range(2):
                for jc in range(NJC):
                    s = slice(jc * D + chalf * 512, jc * D + (chalf + 1) * 512)
                    nc.sync.dma_start(
                        xr[:, s],
                        x_d[
                            b * N + jc * P : b * N + (jc + 1) * P,
                            chalf * 512 : (chalf + 1) * 512,
                        ],
                    )
                    nc.vector.tensor_add(xnb[:, s], xr[:, s], pos_sb[:, s])
                    nc.vector.bn_stats(st4[:, jc, chalf, :], xnb[:, s])
            rstd = small_p.tile([P, NJC], FP, tag="rstd")
            sq = small_p.tile([P, NJC], FP, tag="sq")
            for jc in range(NJC):
                ag = small_p.tile([P, 2], FP, tag="bnag")
                nc.vector.bn_aggr(ag[:, :], st4[:, jc, :, :])
                nc.scalar.activation(
                    sq[:, jc : jc + 1], ag[:, 1:2], ACTF.Sqrt, bias=eps_sb[:, :]
                )
            nc.vector.reciprocal(rstd[:, :], sq[:, :])

            # ---- B: transpose -> xT bf16, kv matmul; emb(b-2) interleaved --
            emb_it = emb_mms(b - 2) if b >= 2 else iter(())
            kvps = ps_wout.tile([P, N], FP, tag="wout", name=f"kvps{b}")
            for chalf in range(2):
                xT = xT_p.tile([P, 4 * N], BF, tag="xT")
                for ccp in range(2):
                    pt = ps_a.tile([P, 2 * N], BF, tag="a")
                    for cci in range(2):
                        cc = chalf * 4 + ccp * 2 + cci
                        for jc in range(NJC):
                            nc.tensor.transpose(
                                pt[:, cci * N + jc * P : cci * N + (jc + 1) * P],
                                xnb[:, jc * D + cc * P : jc * D + (cc + 1) * P],
                                identb[:, :],
                            )
                            mm = next(emb_it, None)
                            if mm is not None:
                                mm()
                    nc.vector.tensor_copy(
                        xT[:, ccp * 2 * N : (ccp + 1) * 2 * N], pt[:, :]
                    )
                for cc4 in range(4):
                    cc = chalf * 4 + cc4
                    nc.tensor.matmul(
                        kvps[:, :],
                        wkv_sb[:, cc * 128 : (cc + 1) * 128],
                        xT[:, cc4 * N : (cc4 + 1) * N],
                        start=(cc == 0),
                        stop=(cc == NCC - 1),
                    )
            kvT = kv_p.tile([P, N], BF, tag="kvT")
            nc.vector.tensor_copy(kvT[:, :], kvps[:, :])
            if b >= 2:
                emit_embT4(b - 2)

            # ---- v natural [j-part, d-free] bf16, scaled by rstd[j] ----
            v_nat = kv_p.tile([P, NJC * (DH + 1)], BF, tag="vnat")
            for jc in range(NJC):
                vt = ps_a.tile([P, DH], BF, tag="a")
                nc.tensor.transpose(
                    vt[:, :],
                    kvT[DH:, jc * P : (jc + 1) * P],
                    identb[DH:P, DH:P],
                )
                nc.vector.tensor_scalar_mul(
                    v_nat[:, jc * 65 : jc * 65 + DH],
                    vt[:, :],
                    rstd[:, jc : jc + 1],
                )
                nc.vector.memset(v_nat[:, jc * 65 + DH : (jc + 1) * 65], 1.0)

            # ---- C: sim + exp per (jc, head-pair); psum double-buffered ----
            es_tiles = []
            for jc in range(NJC):
                es_t = es_p.tile([P, H * N], BF, tag="es")
                es_tiles.append(es_t)
                for hq in range(4):
                    sm = ps_a.tile([P, 2 * N], FP, tag="a")
                    for hh in range(2):
                        h = hq * 2 + hh
                        nc.tensor.matmul(
                            sm[:, hh * N : (hh + 1) * N],
                            kvT[0:DH, jc * P : (jc + 1) * P],
                            qT_sb[:, h * N : (h + 1) * N],
                            start=True,
                            stop=True,
                        )
                    nc.scalar.activation(
                        es_t[:, hq * 2 * N : (hq + 1) * 2 * N],
                        sm[:, :],
                        ACTF.Exp,
                        scale=rstd[:, jc : jc + 1],
                    )

            # ---- E(b-1): wout on PE while scalar drains b's exps ----
            if b >= 1:
                ag2 = emit_E1(b - 1)
                emit_E2(b - 1, ag2)

            # ---- D: attn @ v in 2-head waves; S rides the evacuation ----
            ov4 = []
            ovs[b] = ov4
            s8 = s8_p.tile([8, N], BF, tag="s8")
            for w in range(4):
                vo0 = ps_vo.tile([DH + 1, N], FP, tag="vo")
                vo1 = ps_vo.tile([DH + 1, N], FP, tag="vo")
                for jc in range(NJC):
                    for hh, vo in ((0, vo0), (1, vo1)):
                        h = 2 * w + hh
                        nc.tensor.matmul(
                            vo[:, :],
                            v_nat[:, jc * 65 : (jc + 1) * 65],
                            es_tiles[jc][:, h * N : (h + 1) * N],
                            start=(jc == 0),
                            stop=(jc == NJC - 1),
                        )
                ov = ov_p.tile([P, N], BF, tag="ov", name=f"ov{b}_{w}")
                ov4.append(ov)
                # Even head + its S row (partition 64) in one copy.
                nc.vector.tensor_copy(ov[0 : DH + 1, :], vo0[:, :])
                nc.sync.dma_start(s8[2 * w : 2 * w + 1, :], ov[DH : DH + 1, :])
                ost = ost_p.tile([DH + 1, N], BF, tag="ost")
                nc.vector.tensor_copy(ost[:, :], vo1[:, :])
                nc.sync.dma_start(
                    s8[2 * w + 1 : 2 * w + 2, :], ost[DH : DH + 1, :]
                )
                # Odd v overwrites the S row region (queued after the S DMAs).
                nc.sync.dma_start(ov[DH:P, :], ost[0:DH, :])
            s8f = s8_p.tile([8, N], FP, tag="s8f")
            nc.vector.tensor_copy(s8f[:, :], s8[:, :])
            s8r = s8_p.tile([8, N], FP, tag="s8r")
            nc.vector.reciprocal_approx_fast(s8r[:, :], s8f[:, :])
            for w in range(4):
                rb = ps_vo.tile([P, N], FP, tag="vo")
                nc.tensor.matmul(
                    rb[:, :],
                    e8_sb[:, w * P : (w + 1) * P],
                    s8r[:, :],
                    start=True,
                    stop=True,
                )
                nc.vector.tensor_mul(ov4[w][:, :], ov4[w][:, :], rb[:, :])

        # ---- tail: E(3), emb(2), emb(3), MLP ----
        ag2 = emit_E1(BL - 1)
        emit_E2(BL - 1, ag2)
        for mm in emb_mms(BL - 2):
            mm()
        emit_embT4(BL - 2)
        for mm in emb_mms(BL - 1):
            mm()
        emit_embT4(BL - 1)

        embT4b = consts.tile([P, BL * NCC], BF)
        nc.vector.tensor_copy(embT4b[:, :], embT4[:, :])
        embT4_r = embT4b.rearrange("p (b c) -> p c b", c=NCC)
        for mc in range(NMC):
            hp = ps_a.tile([P, BL], FP, tag="a")
            for cc in range(NCC):
                nc.tensor.matmul(
                    hp[:, :],
                    w1_sb[:, cc * MID + mc * P : cc * MID + (mc + 1) * P],
                    embT4_r[:, cc, :],
                    start=(cc == 0),
                    stop=(cc == NCC - 1),
                )
            gf = ACTF.Gelu if GELU_KIND == "gelu" else ACTF.Identity
            nc.scalar.activation(
                h1_sb[:, mc * BL : (mc + 1) * BL],
                hp[:, :],
                gf,
                bias=b1t_sb[:, mc : mc + 1],
            )
        for pc in range(NPC):
            pp = ps_a.tile([P, BL], FP, tag="a")
            for mc in range(NMC):
                nc.tensor.matmul(
                    pp[:, :],
                    w2_sb[:, mc * PROJ + pc * P : mc * PROJ + (pc + 1) * P],
                    h1_sb[:, mc * BL : (mc + 1) * BL],
                    start=(mc == 0),
                    stop=(mc == NMC - 1),
                )
            nc.vector.tensor_scalar_add(
                pred_sb[:, pc * BL : (pc + 1) * BL], pp[:, :], b2t_sb[:, pc : pc + 1]
            )
            nc.sync.dma_start(
                out_d[pc * P : (pc + 1) * P, :], pred_sb[:, pc * BL : (pc + 1) * BL]
            )

    nc.compile()
    return nc


_NC_CACHE = None


def kernel(**inputs) -> np.ndarray:
    global _NC_CACHE
    from concourse.bass_utils import run_bass_kernel_spmd

    in_maps = _host_prep(inputs)
    if _NC_CACHE is None:
        _NC_CACHE = build_program()
    nc = _NC_CACHE
    res = run_bass_kernel_spmd(nc, in_maps, core_ids=list(range(NCORES)))
    out = np.empty((B, PROJ), np.float32)
    for c in range(NCORES):
        out[c * BL : (c + 1) * BL] = res.results[c]["predT"].T
    return out
